# revision 12
# baseline (speedup 1.0000x reference)
"""Mamba block (dense_transformer nn_Block) on 8 Trainium2 NeuronCores.

Batch-half pipelined schedule. d_inner sharded 8-way (256 ch/core) for
in_proj/conv/scan; x_proj partials AllReduced per batch half; the scan output
is re-sharded to tokens by per-(d,half) AllToAlls; out_proj/MLP run in a
token-transposed layout (tokens on partitions) so LN2 and its application are
pure scalar-engine work and fc/proj use few large matmuls with activations as
the stationary operand. Vector-engine scan of half b overlaps the tensor
engine's out_proj+MLP of half b-1.
"""
import os
import numpy as np
import ml_dtypes

import concourse.bass as bass
import concourse.bacc as bacc
import concourse.mybir as mybir
import concourse.tile as tile
from contextlib import ExitStack
from concourse.bass_utils import run_bass_kernel_spmd

BF16 = mybir.dt.bfloat16
F32 = mybir.dt.float32
AF = mybir.ActivationFunctionType
OP = mybir.AluOpType
bf = ml_dtypes.bfloat16

B, L, E = 2, 1024, 1024
DIN, NST, RDT, KC = 2 * E, 16, 64, 4
EPS = 1e-5
NC = 8
DL = DIN // NC          # 256 channels per core
TOK = B * L             # 2048
TOKB = 128              # tokens per core per batch half
HID = 4 * E             # 4096
G = 4                   # states per scan instruction
NG = NST // G

_BUILD_CACHE = {}


def _rep0(src_ap, parts=128):
    """Partition-broadcast: prepend a [0, parts] dim to an AP's pattern."""
    return bass.AP(src_ap.tensor, src_ap.offset,
                   [[0, parts]] + [list(p) for p in src_ap.ap])


def _build(ln1b_nonzero):
    key = (ln1b_nonzero,)
    if key in _BUILD_CACHE:
        return _BUILD_CACHE[key]

    nc = bacc.Bacc("TRN2", target_bir_lowering=False, debug=False, num_devices=NC)

    def din(name, shape, dt=BF16):
        return nc.dram_tensor(name, shape, dt, kind="ExternalInput").ap()

    xT = din("xT", [E, TOK])
    win = din("win", [128, 8 * 512])
    sw_in = din("sw_in", [1, 512])
    sb_in = din("sb_in", [128, 4], F32)
    convw = din("convw", [128, 2 * KC], F32)
    convb = din("convb", [128, 2], F32)
    xpw = din("xpw", [128, 2 * 96])
    dtw = din("dtw", [64, 256])
    dtb = din("dtb", [128, 2], F32)
    a_sc = din("a_sc", [128, 2 * NST], F32)
    dvec = din("dvec", [128, 2], F32)
    wo = din("wo", [128, 16 * 1024])
    xresT = din("xresT", [2 * TOKB, E], F32)
    wfc = din("wfc", [128, 8 * HID])
    sbfc_row = din("sbfc_row", [1, HID])
    wpj = din("wpj", [128, 32 * E])
    pjb_row = din("pjb_row", [1, E])
    ones128 = din("ones128", [128, 1])
    ident = din("ident", [128, 128])

    outTT = nc.dram_tensor("outTT", [2 * TOKB, E], F32, kind="ExternalOutput").ap()

    cc_dummy_in = nc.dram_tensor("cc_dummy_in", [1, 16], F32)
    cc_dummy_out = nc.dram_tensor("cc_dummy_out", [1, 16], F32, addr_space="Shared")
    ar_ins = [nc.dram_tensor(f"ar_in{b}", [96, L], F32) for b in range(2)]
    ar_outs = [nc.dram_tensor(f"ar_out{b}", [96, L], F32, addr_space="Shared")
               for b in range(2)]
    bc_bfs = [nc.dram_tensor(f"bc_bf{b}", [32, L], BF16) for b in range(2)]
    a2a_ins = {(d, b): nc.dram_tensor(f"a2a_in{d}{b}", [NC, 128 * TOKB], BF16)
               for d in range(2) for b in range(2)}
    a2a_outs = {(d, b): nc.dram_tensor(f"a2a_out{d}{b}", [NC, 128 * TOKB], BF16)
                for d in range(2) for b in range(2)}
    RG = [list(range(NC))]

    with tile.TileContext(nc) as tc, ExitStack() as _stk:
        # warm the collective stream early (absorbs ~80us barrier + delay)
        nc.gpsimd.collective_compute("AllReduce", OP.add, ins=[cc_dummy_in[:]],
                                     outs=[cc_dummy_out[:]], replica_groups=RG)

        cpool = _stk.enter_context(tc.tile_pool(name="consts", bufs=1))
        ones_t = cpool.tile([128, 1], BF16, tag="ones")
        nc.sync.dma_start(ones_t[:], ones128[:])
        ident_t = cpool.tile([128, 128], BF16, tag="ident")
        nc.sync.dma_start(ident_t[:], ident[:])
        ones_row = cpool.tile([1, 128], BF16, tag="onesrow")
        nc.sync.dma_start(ones_row[:], ones128[:].rearrange("p q -> q p"))
        ones_row_f = cpool.tile([1, 128], F32, tag="onesrowf")
        nc.vector.tensor_copy(ones_row_f[:], ones_row[:])
        sw_t = cpool.tile([1, 512], BF16, tag="sw")
        nc.sync.dma_start(sw_t[:], sw_in[:])
        convw_t = cpool.tile([128, 2 * KC], F32, tag="convw")
        nc.sync.dma_start(convw_t[:], convw[:])
        convb_t = cpool.tile([128, 2], F32, tag="convb")
        nc.sync.dma_start(convb_t[:], convb[:])
        xpw_t = cpool.tile([128, 2 * 96], BF16, tag="xpw")
        nc.sync.dma_start(xpw_t[:], xpw[:])
        dtw_t = cpool.tile([64, 256], BF16, tag="dtw")
        nc.sync.dma_start(dtw_t[:], dtw[:])
        dtb_t = cpool.tile([128, 2], F32, tag="dtb")
        nc.sync.dma_start(dtb_t[:], dtb[:])
        asc_t = cpool.tile([128, 2 * NST], F32, tag="asc")
        nc.sync.dma_start(asc_t[:], a_sc[:])
        dvec_t = cpool.tile([128, 2], F32, tag="dvec")
        nc.sync.dma_start(dvec_t[:], dvec[:])
        sbfc_t = cpool.tile([1, HID], BF16, tag="sbfc")
        nc.sync.dma_start(sbfc_t[:], sbfc_row[:])
        pjb_t = cpool.tile([1, E], BF16, tag="pjb")
        nc.sync.dma_start(pjb_t[:], pjb_row[:])
        eps_t = cpool.tile([128, 1], F32, tag="eps")
        nc.vector.memset(eps_t[:], EPS)
        sbin_t = cpool.tile([128, 4], F32, tag="sbin")
        if ln1b_nonzero:
            nc.sync.dma_start(sbin_t[:], sb_in[:])
        xresT_t = [cpool.tile([TOKB, E], F32, tag=f"xresT{b}",
                              name=f"xresT_t{b}") for b in range(2)]
        for b in range(2):
            nc.sync.dma_start(xresT_t[b][:], xresT[b * TOKB:(b + 1) * TOKB, :])

        # ---- mamba-phase activations ----
        # xmp/zt live only within phase_A(b): ring over b.  zs/xs/dt/dtx live
        # until the (d,b) scan completes: separate pool per b so b0's space
        # frees before the MLP phase.
        xmp = [[None, None], [None, None]]
        zt = [[None, None], [None, None]]
        mbp1 = _stk.enter_context(tc.tile_pool(name="mamba1", bufs=1))
        # scan pools: open before the short-lived pools, live to program end
        pa = _stk.enter_context(tc.tile_pool(name="scan_a", bufs=2))
        pbh = _stk.enter_context(tc.tile_pool(name="scan_bh", bufs=3))
        pr = _stk.enter_context(tc.tile_pool(name="scan_r", bufs=2))
        py = _stk.enter_context(tc.tile_pool(name="scan_y", bufs=2))
        ps_y = _stk.enter_context(tc.tile_pool(name="ps_y", bufs=1, space="PSUM"))
        _mb0stk = ExitStack()
        mbp0 = _mb0stk.enter_context(tc.tile_pool(name="mamba0", bufs=1))
        _iostk = ExitStack()
        iop = _iostk.enter_context(tc.tile_pool(name="mamba_io", bufs=2))
        mbp = [mbp0, mbp1]
        zs = [[mbp[b].tile([128, L], BF16, tag=f"zs{d}", name=f"zs{d}{b}")
               for b in range(2)] for d in range(2)]
        xs = [[mbp[b].tile([128, L], BF16, tag=f"xs{d}", name=f"xs{d}{b}")
               for b in range(2)] for d in range(2)]
        dt_t = [[mbp[b].tile([128, L], BF16, tag=f"dt{d}", name=f"dt{d}{b}")
                 for b in range(2)] for d in range(2)]
        dtx = [[mbp[b].tile([128, L], BF16, tag=f"dtx{d}", name=f"dtx{d}{b}")
                for b in range(2)] for d in range(2)]

        # ---- phase-A pools (head + mid1) ----
        _astk = ExitStack()
        p1 = _astk.enter_context(tc.tile_pool(name="ph1", bufs=1))
        p1sq = _astk.enter_context(tc.tile_pool(name="ph1sq", bufs=3))
        ps_st = _astk.enter_context(tc.tile_pool(name="ps_st", bufs=1, space="PSUM"))
        ps_in = _astk.enter_context(tc.tile_pool(name="ps_in", bufs=2, space="PSUM"))
        ps_rb = _astk.enter_context(tc.tile_pool(name="ps_rb", bufs=1, space="PSUM"))
        cvp = _astk.enter_context(tc.tile_pool(name="conv", bufs=2))
        xpp = _astk.enter_context(tc.tile_pool(name="xp", bufs=2))
        ps_xp = _astk.enter_context(tc.tile_pool(name="ps_xp", bufs=1, space="PSUM"))
        dts = _astk.enter_context(tc.tile_pool(name="dts", bufs=2))

        xt = [p1.tile([128, TOK], BF16, tag=f"xt{k}", name=f"xt{k}")
              for k in range(8)]
        for b in range(2):
            for k in range(8):
                nc.sync.dma_start(xt[k][:, b * L:(b + 1) * L],
                                  xT[k * 128:(k + 1) * 128, b * L:(b + 1) * L])
        win_t = p1.tile([128, 8 * 512], BF16, tag="win")
        nc.sync.dma_start(win_t[:], win[:])
        negs = [p1.tile([1, L], BF16, tag=f"negm{b}", name=f"negm{b}")
                for b in range(2)]
        r_reps = [p1.tile([128, L], BF16, tag="r_rep", name=f"r_rep{b}")
                  for b in range(2)]

        def phase_A(b):
            """LN1 stats, in_proj, conv+silu, x_proj partial + AR for half b."""
            for d in range(2):
                xmp[d][b] = iop.tile([128, 3 + L], BF16, tag=f"xmp{d}",
                                     name=f"xmp{d}{b}")
                nc.vector.memset(xmp[d][b][:, 0:3], 0.0)
                zt[d][b] = iop.tile([128, L], BF16, tag=f"z{d}",
                                    name=f"zt{d}{b}")
            # ---- LN1 stats ----
            sum_sb = p1.tile([1, L], F32, tag="rows", bufs=3)
            sq_sb = p1.tile([1, L], F32, tag="rows", bufs=3)
            for ch in range(2):
                sl = slice(b * L + ch * 512, b * L + (ch + 1) * 512)
                dsl = slice(ch * 512, (ch + 1) * 512)
                pss = ps_st.tile([1, 512], F32, tag="pstat", bufs=2)
                for k in range(8):
                    nc.tensor.matmul(pss[:], ones_t[:], xt[k][:, sl],
                                     start=(k == 0), stop=(k == 7))
                nc.vector.tensor_copy(sum_sb[:, dsl], pss[:])
                psq = ps_st.tile([1, 512], F32, tag="pstat", bufs=2)
                for k in range(8):
                    xq = p1sq.tile([128, 512], BF16, tag="xq", bufs=2)
                    nc.scalar.activation(xq[:], xt[k][:, sl], AF.Square)
                    nc.tensor.matmul(psq[:], ones_t[:], xq[:],
                                     start=(k == 0), stop=(k == 7))
                nc.vector.tensor_copy(sq_sb[:, dsl], psq[:])
            m_neg = p1.tile([1, L], F32, tag="rows", bufs=3)
            nc.vector.tensor_scalar_mul(m_neg[:], sum_sb[:], -1.0 / E)
            nc.vector.tensor_copy(negs[b][:], m_neg[:])
            msq = p1.tile([1, L], F32, tag="rows", bufs=3)
            nc.vector.tensor_tensor(msq[:], m_neg[:], m_neg[:], OP.mult)
            var = p1.tile([1, L], F32, tag="rows", bufs=3)
            nc.vector.scalar_tensor_tensor(var[:], sq_sb[:], 1.0 / E,
                                           msq[:], OP.mult, OP.subtract)
            lnv = p1.tile([1, L], F32, tag="rows", bufs=3)
            nc.scalar.activation(lnv[:], var[:], AF.Ln, bias=eps_t[0:1, :])
            r_sb = p1.tile([1, L], F32, tag="rows", bufs=3)
            nc.scalar.activation(r_sb[:], lnv[:], AF.Exp, scale=-0.5)
            for hh in range(2):
                prb = ps_rb.tile([128, 512], F32, tag="prb")
                nc.tensor.matmul(prb[:], ones_row_f[:],
                                 r_sb[:, hh * 512:(hh + 1) * 512],
                                 start=True, stop=True)
                nc.vector.tensor_copy(r_reps[b][:, hh * 512:(hh + 1) * 512],
                                      prb[:])

            # ---- in_proj (xm tiles first, z tiles after conv) ----
            def inproj_tile(mt, ch):
                col = ch * 512
                sl = slice(b * L + col, b * L + col + 512)
                ps = ps_in.tile([128, 512], F32, tag="ps", bufs=2)
                for k in range(8):
                    nc.tensor.matmul(
                        ps[:],
                        win_t[:, k * 512 + mt * 128:k * 512 + (mt + 1) * 128],
                        xt[k][:, sl], start=(k == 0), stop=False)
                nc.tensor.matmul(ps[:], sw_t[:, mt * 128:(mt + 1) * 128],
                                 negs[b][:, col:col + 512],
                                 start=False, stop=True)
                if mt < 2:
                    dst = xmp[mt][b][:, 3 + col:3 + col + 512]
                else:
                    dst = zt[mt - 2][b][:, col:col + 512]
                if ln1b_nonzero:
                    tmp = p1sq.tile([128, 512], F32, tag="eptmp")
                    nc.vector.tensor_tensor(tmp[:], ps[:],
                                            r_reps[b][:, col:col + 512],
                                            OP.mult)
                    nc.scalar.activation(dst, tmp[:], AF.Identity,
                                         bias=sbin_t[:, mt:mt + 1])
                else:
                    nc.vector.tensor_tensor(dst, ps[:],
                                            r_reps[b][:, col:col + 512],
                                            OP.mult)

            for mt in range(2):
                for ch in range(2):
                    inproj_tile(mt, ch)

            # ---- conv + silu ----
            for d in range(2):
                acc0 = cvp.tile([128, L], BF16, tag="acc", bufs=2)
                nc.vector.tensor_scalar_mul(acc0[:], xmp[d][b][:, 0:L],
                                            convw_t[:, d * KC:d * KC + 1])
                for k in (1, 2, 3):
                    acc1 = cvp.tile([128, L], BF16, tag="acc", bufs=2)
                    nc.vector.scalar_tensor_tensor(
                        acc1[:], xmp[d][b][:, k:k + L],
                        convw_t[:, d * KC + k:d * KC + k + 1],
                        acc0[:], OP.mult, OP.add)
                    acc0 = acc1
                nc.scalar.activation(xs[d][b][:], acc0[:], AF.Silu,
                                     bias=convb_t[:, d:d + 1])

            # z tiles + silu(z)
            for mt in range(2, 4):
                for ch in range(2):
                    inproj_tile(mt, ch)
            for d in range(2):
                nc.scalar.activation(zs[d][b][:], zt[d][b][:], AF.Silu)

            # ---- x_proj partial + AllReduce ----
            xdblp = xpp.tile([96, L], F32, tag="xdblp", bufs=1)
            for ch in range(2):
                col = ch * 512
                psx = ps_xp.tile([96, 512], F32, tag="psx")
                for k in range(2):
                    nc.tensor.matmul(psx[:], xpw_t[:, k * 96:(k + 1) * 96],
                                     xs[k][b][:, col:col + 512],
                                     start=(k == 0), stop=(k == 1))
                nc.vector.tensor_copy(xdblp[:, col:col + 512], psx[:])
            nc.sync.dma_start(ar_ins[b][:], xdblp[:])
            nc.gpsimd.collective_compute("AllReduce", OP.add,
                                         ins=[ar_ins[b][:]],
                                         outs=[ar_outs[b][:]],
                                         replica_groups=RG)

        def phase_dt(b):
            """dt softplus + dtx for half b (after AR(b))."""
            dtr_b = dts.tile([64, L], BF16, tag="dtrb", bufs=1)
            nc.gpsimd.dma_start(dtr_b[:], ar_outs[b][0:64, :])
            nc.gpsimd.dma_start(bc_bfs[b][:], ar_outs[b][64:96, :])
            for mt in range(2):
                dte_t = dts.tile([128, L], BF16, tag="dte")
                for ch in range(2):
                    col = ch * 512
                    psd = ps_in.tile([128, 512], F32, tag="ps", bufs=2)
                    nc.tensor.matmul(psd[:],
                                     dtw_t[:, mt * 128:(mt + 1) * 128],
                                     dtr_b[:, col:col + 512],
                                     start=True, stop=True)
                    nc.scalar.activation(dte_t[:, col:col + 512],
                                         psd[:], AF.Exp,
                                         bias=dtb_t[:, mt:mt + 1])
                nc.scalar.activation(dt_t[mt][b][:], dte_t[:],
                                     AF.Ln, bias=1.0)
                nc.vector.tensor_tensor(dtx[mt][b][:], dt_t[mt][b][:],
                                        xs[mt][b][:], OP.mult)

        psy_cur = [None]

        def scan_group(d, b, g):
            if g == 0:
                psy_cur[0] = ps_y.tile([128, L], F32, tag="psy", name="psy")
            psy = psy_cur[0]
            a_t = pa.tile([128, G, L], BF16, tag="a")
            for j in range(G):
                n = g * G + j
                nc.scalar.activation(
                    a_t[:, j, :], dt_t[d][b][:], AF.Exp,
                    scale=asc_t[:, d * NST + n:d * NST + n + 1])
            nc.vector.memset(a_t[:, :, 0:1], 0.0)
            brep = pr.tile([128, G, L], BF16, tag="bcr")
            nc.sync.dma_start(brep[:],
                              _rep0(bc_bfs[b][g * G:(g + 1) * G, :]))
            bx = pbh.tile([128, G, L], BF16, tag="bxhc")
            dslice = dtx[d][b][:]
            dxb = bass.AP(dslice.tensor, dslice.offset,
                          [list(dslice.ap[0]), [0, G], [1, L]])
            nc.vector.tensor_tensor(bx[:], dxb, brep[:], OP.mult)
            h_t = pbh.tile([128, G, L], BF16, tag="bxhc")
            nc.vector.tensor_tensor_scan(
                h_t[:].rearrange("p a b -> p (a b)"),
                a_t[:].rearrange("p a b -> p (a b)"),
                bx[:].rearrange("p a b -> p (a b)"),
                0.0, OP.mult, OP.add)
            crep = pr.tile([128, G, L], BF16, tag="bcr")
            nc.sync.dma_start(crep[:],
                              _rep0(bc_bfs[b][16 + g * G:16 + (g + 1) * G, :]))
            hc = pbh.tile([128, G, L], BF16, tag="bxhc")
            nc.vector.tensor_tensor(hc[:], h_t[:], crep[:], OP.mult)
            for j in range(G):
                for hh in range(2):
                    nc.tensor.matmul(
                        psy[:, hh * 512:(hh + 1) * 512], ident_t[:],
                        hc[:, j, hh * 512:(hh + 1) * 512],
                        start=(g == 0 and j == 0),
                        stop=(g == NG - 1 and j == G - 1))

        def scan_fin(d, b):
            """y2/y3 + A2A staging + launch A2A(d, b)."""
            psy = psy_cur[0]
            y2 = py.tile([128, L], BF16, tag="y2")
            nc.vector.scalar_tensor_tensor(y2[:], xs[d][b][:],
                                           dvec_t[:, d:d + 1], psy[:],
                                           OP.mult, OP.add)
            y3 = py.tile([128, L], BF16, tag="y2")
            nc.vector.tensor_tensor(y3[:], y2[:], zs[d][b][:], OP.mult)
            for c in range(NC):
                nc.sync.dma_start(
                    a2a_ins[(d, b)][c].rearrange("(p q) -> p q", p=128),
                    y3[:, c * TOKB:(c + 1) * TOKB])
            nc.gpsimd.collective_compute("AllToAll", OP.bypass,
                                         ins=[a2a_ins[(d, b)][:]],
                                         outs=[a2a_outs[(d, b)][:]],
                                         replica_groups=RG)

        # ================= emission: head =================
        phase_A(0)
        phase_dt(0)

        # ================= emission: mid1 =================
        scan_group(0, 0, 0)
        scan_group(0, 0, 1)
        phase_A(1)
        scan_group(0, 0, 2)
        scan_group(0, 0, 3)
        scan_fin(0, 0)
        scan_group(1, 0, 0)
        scan_group(1, 0, 1)
        phase_dt(1)
        scan_group(1, 0, 2)
        scan_group(1, 0, 3)
        scan_fin(1, 0)

        _astk.close()   # frees xt/win pools + phase-A PSUM
        _iostk.close()  # frees xmp/zt rings
        _mb0stk.close()  # frees b0 scan activations

        # MLP-side pools + weight tiles (wo resident; wfc/wpj streamed)
        mlpp = _stk.enter_context(tc.tile_pool(name="mlp", bufs=1, side="right"))
        wo_t = mlpp.tile([128, 16 * 1024], BF16, tag="wo")
        nc.sync.dma_start(wo_t[:], wo[:])

        opool = _stk.enter_context(tc.tile_pool(name="opool", bufs=1))
        wfcp = _stk.enter_context(tc.tile_pool(name="wfcp", bufs=6, side="right"))
        wpjp = _stk.enter_context(tc.tile_pool(name="wpjp", bufs=2, side="right"))

        def phase_O(b, interleave=None):
            """out_proj + LN2 + MLP for half b's 128 tokens (transposed layout).

            interleave: optional list of (after_step, fn) to emit scan work
            between tensor-heavy steps.
            """
            steps = dict(interleave or [])

            def run(tag):
                if tag in steps:
                    steps[tag]()

            with tc.tile_pool(name="ps_op", bufs=1, space="PSUM") as ps_op, \
                 tc.tile_pool(name="yfp", bufs=1) as yfp:
                r1T_ps = ps_op.tile([128, E], F32, tag="r1T")
                yf = {}
                for d in range(2):
                    for i in range(NC):
                        t = yfp.tile([128, TOKB], BF16, tag=f"yf{d}_{i}")
                        nc.sync.dma_start(
                            t[:],
                            a2a_outs[(d, b)][i].rearrange("(p q) -> p q", p=128))
                        yf[(d, i)] = t
                run("yf")
                for d in range(2):
                    for i in range(NC):
                        ci = i * 2 + d
                        for hh in range(2):
                            nc.tensor.matmul(
                                r1T_ps[:, hh * 512:(hh + 1) * 512],
                                yf[(d, i)][:],
                                wo_t[:, ci * 1024 + hh * 512:
                                     ci * 1024 + (hh + 1) * 512],
                                start=(d == 0 and i == 0),
                                stop=(d == 1 and i == NC - 1),
                                skip_group_check=True)
                run("op")
                r1fT = opool.tile([128, E], F32, tag="r1fT", bufs=1)
                nc.vector.tensor_tensor(r1fT[:], r1T_ps[:], xresT_t[b][:],
                                        OP.add)

            # LN2 stats on scalar engine (per-partition = per-token)
            scr = opool.tile([128, E], F32, tag="scr", bufs=1)
            s1 = opool.tile([128, 1], F32, tag="s1", bufs=2)
            s2 = opool.tile([128, 1], F32, tag="s2", bufs=2)
            nc.scalar.activation(scr[:], r1fT[:], AF.Identity, accum_out=s1[:])
            nc.scalar.activation(scr[:], r1fT[:], AF.Square, accum_out=s2[:])
            m_neg = opool.tile([128, 1], F32, tag="mneg", bufs=2)
            nc.vector.tensor_scalar_mul(m_neg[:], s1[:], -1.0 / E)
            msq = opool.tile([128, 1], F32, tag="msq", bufs=2)
            nc.vector.tensor_tensor(msq[:], m_neg[:], m_neg[:], OP.mult)
            var = opool.tile([128, 1], F32, tag="var", bufs=2)
            nc.vector.scalar_tensor_tensor(var[:], s2[:], 1.0 / E, msq[:],
                                           OP.mult, OP.subtract)
            lnv = opool.tile([128, 1], F32, tag="lnv", bufs=2)
            nc.scalar.activation(lnv[:], var[:], AF.Ln, bias=eps_t[:])
            rstd = opool.tile([128, 1], F32, tag="rstd", bufs=2)
            nc.scalar.activation(rstd[:], lnv[:], AF.Exp, scale=-0.5)
            nmb = opool.tile([128, 1], F32, tag="nmb", bufs=2)
            nc.vector.tensor_tensor(nmb[:], m_neg[:], rstd[:], OP.mult)
            r1nT = opool.tile([128, E], BF16, tag="r1nT", bufs=2)
            nc.scalar.activation(r1nT[:], r1fT[:], AF.Identity,
                                 bias=nmb[:], scale=rstd[:])
            # transpose r1nT chunks for fc stationary
            r1tt = opool.tile([128, E], BF16, tag="r1tt", bufs=2)
            for k in range(8):
                nc.sync.dma_start_transpose(r1tt[:, k * 128:(k + 1) * 128],
                                            r1nT[:, k * 128:(k + 1) * 128])
            run("r1")

            # fc: psf[tok, hid-quarter]; wfc streamed; gelu on scalar
            h1sT = opool.tile([128, HID], BF16, tag="h1sT", bufs=1,
                              name=f"h1sT{b}")
            with tc.tile_pool(name="ps_fc", bufs=2, space="PSUM") as ps_fc:
                for q in range(4):
                    psf = ps_fc.tile([128, 1024], F32, tag="psf", bufs=2)
                    for k in range(8):
                        wpc = wfcp.tile([128, 1024], BF16, tag="wfc",
                                        name=f"wfc{b}_{q}_{k}")
                        nc.sync.dma_start(
                            wpc[:], wfc[:, k * HID + q * 1024:
                                         k * HID + (q + 1) * 1024])
                        for hh in range(2):
                            nc.tensor.matmul(
                                psf[:, hh * 512:(hh + 1) * 512],
                                r1tt[:, k * 128:(k + 1) * 128],
                                wpc[:, hh * 512:(hh + 1) * 512],
                                start=(k == 0), stop=False,
                                skip_group_check=True)
                    for hh in range(2):
                        nc.tensor.matmul(
                            psf[:, hh * 512:(hh + 1) * 512], ones_row[:],
                            sbfc_t[:, q * 1024 + hh * 512:
                                   q * 1024 + (hh + 1) * 512],
                            start=False, stop=True, skip_group_check=True)
                    nc.scalar.activation(h1sT[:, q * 1024:(q + 1) * 1024],
                                         psf[:], AF.Gelu)
                    if q in (1, 3):
                        run(f"fc{q // 2}")

            # transpose h1 chunks; proj with streamed wpj
            h1tt = opool.tile([128, HID], BF16, tag="h1tt", bufs=1,
                              name=f"h1tt{b}")
            for j in range(32):
                nc.sync.dma_start_transpose(h1tt[:, j * 128:(j + 1) * 128],
                                            h1sT[:, j * 128:(j + 1) * 128])
            run("h1t")
            with tc.tile_pool(name="ps_pj", bufs=1, space="PSUM") as ps_pj:
                psp = ps_pj.tile([128, E], F32, tag="psp")
                for jg in range(8):
                    wpj_g = wpjp.tile([128, 4 * E], BF16, tag="wpjg",
                                      name=f"wpj{b}_{jg}")
                    nc.sync.dma_start(wpj_g[:],
                                      wpj[:, jg * 4 * E:(jg + 1) * 4 * E])
                    for jj in range(4):
                        j = jg * 4 + jj
                        for hh in range(2):
                            nc.tensor.matmul(
                                psp[:, hh * 512:(hh + 1) * 512],
                                h1tt[:, j * 128:(j + 1) * 128],
                                wpj_g[:, jj * E + hh * 512:jj * E + (hh + 1) * 512],
                                start=(j == 0), stop=False,
                                skip_group_check=True)
                for hh in range(2):
                    nc.tensor.matmul(psp[:, hh * 512:(hh + 1) * 512],
                                     ones_row[:],
                                     pjb_t[:, hh * 512:(hh + 1) * 512],
                                     start=False, stop=True,
                                     skip_group_check=True)
                run("pj")
                ot = opool.tile([128, E], F32, tag="ot", bufs=1)
                nc.vector.tensor_tensor(ot[:], psp[:], r1fT[:], OP.add)
                nc.sync.dma_start(outTT[b * TOKB:(b + 1) * TOKB, :], ot[:])

        # ================= emission: mid2 (scan b1 || O(b0)) =================
        sg = scan_group
        phase_O(0, interleave=[
            ("yf", lambda: sg(0, 1, 0)),
            ("op", lambda: sg(0, 1, 1)),
            ("r1", lambda: sg(0, 1, 2)),
            ("fc0", lambda: (sg(0, 1, 3), scan_fin(0, 1))),
            ("fc1", lambda: (sg(1, 1, 0), sg(1, 1, 1))),
            ("h1t", lambda: sg(1, 1, 2)),
            ("pj", lambda: (sg(1, 1, 3), scan_fin(1, 1))),
        ])

        # ================= emission: tail (O(b1)) =================
        phase_O(1)

    nc.compile()
    _BUILD_CACHE[key] = nc
    return nc


def _prep_inputs(inputs):
    """Host-side sharding/packing. Returns list of per-core input dicts."""
    f32 = np.float32
    x = np.asarray(inputs["x"], f32)
    ln1_w = np.asarray(inputs["ln1_w"], f32)
    ln1_b = np.asarray(inputs["ln1_b"], f32)
    in_proj_w = np.asarray(inputs["in_proj_w"], f32)
    conv_w = np.asarray(inputs["conv_w"], f32)
    conv_b = np.asarray(inputs["conv_b"], f32)
    x_proj_w = np.asarray(inputs["x_proj_w"], f32)
    dt_proj_w = np.asarray(inputs["dt_proj_w"], f32)
    dt_proj_b = np.asarray(inputs["dt_proj_b"], f32)
    A_log = np.asarray(inputs["A_log"], f32)
    D = np.asarray(inputs["D"], f32)
    out_proj_w = np.asarray(inputs["out_proj_w"], f32)
    ln2_w = np.asarray(inputs["ln2_w"], f32)
    ln2_b = np.asarray(inputs["ln2_b"], f32)
    fc_w = np.asarray(inputs["fc_w"], f32)
    fc_b = np.asarray(inputs["fc_b"], f32)
    proj_w = np.asarray(inputs["proj_w"], f32)
    proj_b = np.asarray(inputs["proj_b"], f32)

    x2d = np.ascontiguousarray(x.reshape(TOK, E))             # [TOK, E]
    xT_b = np.ascontiguousarray(x2d.T).astype(bf)             # [E, TOK]

    def pack_lhsT(lhsT):
        K, M = lhsT.shape
        nk = K // 128
        return np.ascontiguousarray(
            lhsT.reshape(nk, 128, M).transpose(1, 0, 2).reshape(128, nk * M)
        ).astype(bf)

    Wp = in_proj_w * ln1_w[None, :]
    sb_full = in_proj_w @ ln1_b
    ln1b_nonzero = bool(np.any(sb_full != 0.0))

    Wfc = fc_w * ln2_w[None, :]
    sbfc_full = fc_w @ ln2_b + fc_b
    wfc_pack = pack_lhsT(np.ascontiguousarray(Wfc.T))
    wpj_pack = pack_lhsT(np.ascontiguousarray(proj_w.T))
    woT = np.ascontiguousarray(out_proj_w.T)                  # [DIN, E]
    wo_pack = np.ascontiguousarray(np.hstack(
        [woT[ci * 128:(ci + 1) * 128, :] for ci in range(16)])).astype(bf)
    sbfc_row = sbfc_full[None, :].astype(bf)
    pjb_row = proj_b[None, :].astype(bf)

    A = -np.exp(A_log)

    per_core = []
    for c in range(NC):
        dsl = slice(c * DL, (c + 1) * DL)
        rows = np.concatenate([Wp[dsl], Wp[DIN + c * DL:DIN + (c + 1) * DL]])
        win_pack = pack_lhsT(np.ascontiguousarray(rows.T))
        sw_row = rows.sum(1)[None, :].astype(bf)
        sb_rows = np.concatenate([sb_full[dsl],
                                  sb_full[DIN + c * DL:DIN + (c + 1) * DL]])
        sb_pack = np.ascontiguousarray(sb_rows.reshape(4, 128).T).astype(f32)

        cw = conv_w[dsl, 0, :]
        convw_pack = np.ascontiguousarray(
            cw.reshape(2, 128, KC).transpose(1, 0, 2).reshape(128, 2 * KC)
        ).astype(f32)
        convb_pack = np.ascontiguousarray(
            conv_b[dsl].reshape(2, 128).T).astype(f32)

        xpw_pack = pack_lhsT(np.ascontiguousarray(x_proj_w[:, dsl].T))
        dtw_slice = np.ascontiguousarray(dt_proj_w[dsl].T).astype(bf)
        dtb_pack = np.ascontiguousarray(
            dt_proj_b[dsl].reshape(2, 128).T).astype(f32)
        asc_pack = np.ascontiguousarray(
            A[dsl].reshape(2, 128, NST).transpose(1, 0, 2).reshape(128, 2 * NST)
        ).astype(f32)
        dvec_pack = np.ascontiguousarray(D[dsl].reshape(2, 128).T).astype(f32)

        xresT_slice = np.ascontiguousarray(np.concatenate(
            [x2d[c * TOKB:(c + 1) * TOKB, :],
             x2d[L + c * TOKB:L + (c + 1) * TOKB, :]]))       # [256, E]

        per_core.append({
            "xT": xT_b, "win": win_pack, "sw_in": sw_row, "sb_in": sb_pack,
            "convw": convw_pack, "convb": convb_pack, "xpw": xpw_pack,
            "dtw": dtw_slice, "dtb": dtb_pack, "a_sc": asc_pack,
            "dvec": dvec_pack, "wo": wo_pack, "xresT": xresT_slice,
            "wfc": wfc_pack, "sbfc_row": sbfc_row,
            "wpj": wpj_pack, "pjb_row": pjb_row,
            "ones128": np.ones((128, 1), bf),
            "ident": np.eye(128, dtype=bf),
        })
    return per_core, ln1b_nonzero


def kernel(**inputs):
    per_core, ln1b_nonzero = _prep_inputs(inputs)
    nc = _build(ln1b_nonzero)
    trace = bool(int(os.environ.get("BASSK_TRACE", "0")))
    try:
        res = run_bass_kernel_spmd(nc, per_core, core_ids=list(range(NC)),
                                   trace=trace)
    except Exception:
        # transient device hiccups (e.g. NRT exec-unit errors) clear on retry
        res = run_bass_kernel_spmd(nc, per_core, core_ids=list(range(NC)),
                                   trace=trace)
    kernel.last_results = res
    out2d = np.empty((TOK, E), np.float32)
    for c in range(NC):
        r = res.results[c]["outTT"]
        out2d[c * TOKB:(c + 1) * TOKB] = r[:TOKB]
        out2d[L + c * TOKB:L + (c + 1) * TOKB] = r[TOKB:]
    return out2d.reshape(B, L, E).astype(np.float32)


# revision 15
# speedup vs baseline: 1.1049x; 1.1049x over previous
"""Mamba block (dense_transformer nn_Block) on 8 Trainium2 NeuronCores.

Batch-half pipelined schedule. d_inner sharded 8-way (256 ch/core) for
in_proj/conv/scan; x_proj partials AllReduced per batch half; the scan output
is re-sharded to tokens by per-(d,half) AllToAlls; out_proj runs in a
token-transposed layout (tokens on partitions) so LN2 and its application are
pure scalar-engine work; fc consumes the transposed-back activations with
streamed weights and its output tiles feed proj directly as the stationary
operand.  The vector-engine scan of half b overlaps the tensor engine's
out_proj+MLP of half b-1.
"""
import os
import numpy as np
import ml_dtypes

import concourse.bass as bass
import concourse.bacc as bacc
import concourse.mybir as mybir
import concourse.tile as tile
from contextlib import ExitStack
from concourse.bass_utils import run_bass_kernel_spmd

BF16 = mybir.dt.bfloat16
F32 = mybir.dt.float32
AF = mybir.ActivationFunctionType
OP = mybir.AluOpType
bf = ml_dtypes.bfloat16

B, L, E = 2, 1024, 1024
DIN, NST, RDT, KC = 2 * E, 16, 64, 4
EPS = 1e-5
NC = 8
DL = DIN // NC          # 256 channels per core
TOK = B * L             # 2048
TOKB = 128              # tokens per core per batch half
HID = 4 * E             # 4096
G = 4                   # states per scan instruction
NG = NST // G

_BUILD_CACHE = {}


def _rep0(src_ap, parts=128):
    """Partition-broadcast: prepend a [0, parts] dim to an AP's pattern."""
    return bass.AP(src_ap.tensor, src_ap.offset,
                   [[0, parts]] + [list(p) for p in src_ap.ap])


def _build(ln1b_nonzero):
    key = (ln1b_nonzero,)
    if key in _BUILD_CACHE:
        return _BUILD_CACHE[key]

    nc = bacc.Bacc("TRN2", target_bir_lowering=False, debug=False, num_devices=NC)

    def din(name, shape, dt=BF16):
        return nc.dram_tensor(name, shape, dt, kind="ExternalInput").ap()

    xT = din("xT", [E, TOK])
    win = din("win", [128, 8 * 512])
    sw_in = din("sw_in", [1, 512])
    sb_in = din("sb_in", [128, 4], F32)
    convw = din("convw", [128, 2 * KC], F32)
    convb = din("convb", [128, 2], F32)
    xpw = din("xpw", [128, 2 * 96])
    dtw = din("dtw", [64, 256])
    dtb = din("dtb", [128, 2], F32)
    a_sc = din("a_sc", [128, 2 * NST], F32)
    dvec = din("dvec", [128, 2], F32)
    wo = din("wo", [128, 16 * 1024])
    xresT = din("xresT", [2 * TOKB, E], F32)
    wfc = din("wfc", [128, 8 * HID])
    sbfc = din("sbfc", [128, 32], F32)
    wpj = din("wpj", [128, 32 * E])
    pjb_row = din("pjb_row", [1, E])
    ones128 = din("ones128", [128, 1])
    ident = din("ident", [128, 128])

    outTT = nc.dram_tensor("outTT", [2 * TOKB, E], F32, kind="ExternalOutput").ap()

    cc_dummy_in = nc.dram_tensor("cc_dummy_in", [1, 16], F32)
    cc_dummy_out = nc.dram_tensor("cc_dummy_out", [1, 16], F32, addr_space="Shared")
    ar_ins = [nc.dram_tensor(f"ar_in{b}", [96, L], F32) for b in range(2)]
    ar_outs = [nc.dram_tensor(f"ar_out{b}", [96, L], F32, addr_space="Shared")
               for b in range(2)]
    bc_bfs = [nc.dram_tensor(f"bc_bf{b}", [32, L], BF16) for b in range(2)]
    a2a_ins = {(d, b): nc.dram_tensor(f"a2a_in{d}{b}", [NC, 128 * TOKB], BF16)
               for d in range(2) for b in range(2)}
    a2a_outs = {(d, b): nc.dram_tensor(f"a2a_out{d}{b}", [NC, 128 * TOKB], BF16)
                for d in range(2) for b in range(2)}
    RG = [list(range(NC))]

    with tile.TileContext(nc) as tc, ExitStack() as _stk:
        # warm the collective stream early (absorbs ~80us barrier + delay)
        nc.gpsimd.collective_compute("AllReduce", OP.add, ins=[cc_dummy_in[:]],
                                     outs=[cc_dummy_out[:]], replica_groups=RG)

        cpool = _stk.enter_context(tc.tile_pool(name="consts", bufs=1))
        ones_t = cpool.tile([128, 1], BF16, tag="ones")
        nc.sync.dma_start(ones_t[:], ones128[:])
        ident_t = cpool.tile([128, 128], BF16, tag="ident")
        nc.sync.dma_start(ident_t[:], ident[:])
        ones_row = cpool.tile([1, 128], BF16, tag="onesrow")
        nc.sync.dma_start(ones_row[:], ones128[:].rearrange("p q -> q p"))
        ones_row_f = cpool.tile([1, 128], F32, tag="onesrowf")
        nc.vector.tensor_copy(ones_row_f[:], ones_row[:])
        sw_t = cpool.tile([1, 512], BF16, tag="sw")
        nc.sync.dma_start(sw_t[:], sw_in[:])
        convw_t = cpool.tile([128, 2 * KC], F32, tag="convw")
        nc.sync.dma_start(convw_t[:], convw[:])
        convb_t = cpool.tile([128, 2], F32, tag="convb")
        nc.sync.dma_start(convb_t[:], convb[:])
        xpw_t = cpool.tile([128, 2 * 96], BF16, tag="xpw")
        nc.sync.dma_start(xpw_t[:], xpw[:])
        dtw_t = cpool.tile([64, 256], BF16, tag="dtw")
        nc.sync.dma_start(dtw_t[:], dtw[:])
        dtb_t = cpool.tile([128, 2], F32, tag="dtb")
        nc.sync.dma_start(dtb_t[:], dtb[:])
        asc_t = cpool.tile([128, 2 * NST], F32, tag="asc")
        nc.sync.dma_start(asc_t[:], a_sc[:])
        dvec_t = cpool.tile([128, 2], F32, tag="dvec")
        nc.sync.dma_start(dvec_t[:], dvec[:])
        sbfc_t = cpool.tile([128, 32], F32, tag="sbfc")
        nc.sync.dma_start(sbfc_t[:], sbfc[:])
        pjb_t = cpool.tile([1, E], BF16, tag="pjb")
        nc.sync.dma_start(pjb_t[:], pjb_row[:])
        eps_t = cpool.tile([128, 1], F32, tag="eps")
        nc.vector.memset(eps_t[:], EPS)
        sbin_t = cpool.tile([128, 4], F32, tag="sbin")
        if ln1b_nonzero:
            nc.sync.dma_start(sbin_t[:], sb_in[:])
        xresT_t = [cpool.tile([TOKB, E], F32, tag=f"xresT{b}",
                              name=f"xresT_t{b}") for b in range(2)]
        for b in range(2):
            nc.sync.dma_start(xresT_t[b][:], xresT[b * TOKB:(b + 1) * TOKB, :])

        # ---- long-lived pools (to program end) ----
        xmp = [[None, None], [None, None]]
        zt = [[None, None], [None, None]]
        mbp1 = _stk.enter_context(tc.tile_pool(name="mamba1", bufs=1))
        pa = _stk.enter_context(tc.tile_pool(name="scan_a", bufs=2))
        pbh = _stk.enter_context(tc.tile_pool(name="scan_bh", bufs=4))
        pr = _stk.enter_context(tc.tile_pool(name="scan_r", bufs=2))
        py = _stk.enter_context(tc.tile_pool(name="scan_y", bufs=2))
        ps_y = _stk.enter_context(tc.tile_pool(name="ps_y", bufs=1, space="PSUM"))
        # ---- short-lived pools (close at mid2 start) ----
        _mb0stk = ExitStack()
        mbp0 = _mb0stk.enter_context(tc.tile_pool(name="mamba0", bufs=1))
        _iostk = ExitStack()
        iop = _iostk.enter_context(tc.tile_pool(name="mamba_io", bufs=2))
        mbp = [mbp0, mbp1]
        zs = [[mbp[b].tile([128, L], BF16, tag=f"zs{d}", name=f"zs{d}{b}")
               for b in range(2)] for d in range(2)]
        xs = [[mbp[b].tile([128, L], BF16, tag=f"xs{d}", name=f"xs{d}{b}")
               for b in range(2)] for d in range(2)]
        dt_t = [[mbp[b].tile([128, L], BF16, tag=f"dt{d}", name=f"dt{d}{b}")
                 for b in range(2)] for d in range(2)]
        dtx = [[mbp[b].tile([128, L], BF16, tag=f"dtx{d}", name=f"dtx{d}{b}")
                for b in range(2)] for d in range(2)]

        # ---- phase-A pools (head + mid1) ----
        _astk = ExitStack()
        p1 = _astk.enter_context(tc.tile_pool(name="ph1", bufs=1))
        p1sq = _astk.enter_context(tc.tile_pool(name="ph1sq", bufs=3))
        ps_st = _astk.enter_context(tc.tile_pool(name="ps_st", bufs=1, space="PSUM"))
        ps_in = _astk.enter_context(tc.tile_pool(name="ps_in", bufs=2, space="PSUM"))
        ps_rb = _astk.enter_context(tc.tile_pool(name="ps_rb", bufs=1, space="PSUM"))
        cvp = _astk.enter_context(tc.tile_pool(name="conv", bufs=2))
        xpp = _astk.enter_context(tc.tile_pool(name="xp", bufs=2))
        ps_xp = _astk.enter_context(tc.tile_pool(name="ps_xp", bufs=1, space="PSUM"))
        dts = _astk.enter_context(tc.tile_pool(name="dts", bufs=2))

        xt = [p1.tile([128, TOK], BF16, tag=f"xt{k}", name=f"xt{k}")
              for k in range(8)]
        for k in range(8):
            nc.sync.dma_start(xt[k][:], xT[k * 128:(k + 1) * 128, :])
        win_t = p1.tile([128, 8 * 512], BF16, tag="win")
        nc.sync.dma_start(win_t[:], win[:])
        negs = [p1.tile([1, L], BF16, tag=f"negm{b}", name=f"negm{b}")
                for b in range(2)]
        r_reps = [p1.tile([128, L], BF16, tag="r_rep", name=f"r_rep{b}")
                  for b in range(2)]

        def phase_A(b):
            """LN1 stats, in_proj, conv+silu, x_proj partial + AR for half b."""
            for d in range(2):
                xmp[d][b] = iop.tile([128, 3 + L], BF16, tag=f"xmp{d}",
                                     name=f"xmp{d}{b}")
                nc.vector.memset(xmp[d][b][:, 0:3], 0.0)
                zt[d][b] = iop.tile([128, L], BF16, tag=f"z{d}",
                                    name=f"zt{d}{b}")
            # ---- LN1 stats ----
            sum_sb = p1.tile([1, L], F32, tag="rows", bufs=3)
            sq_sb = p1.tile([1, L], F32, tag="rows", bufs=3)
            for ch in range(2):
                sl = slice(b * L + ch * 512, b * L + (ch + 1) * 512)
                dsl = slice(ch * 512, (ch + 1) * 512)
                pss = ps_st.tile([1, 512], F32, tag="pstat", bufs=2)
                for k in range(8):
                    nc.tensor.matmul(pss[:], ones_t[:], xt[k][:, sl],
                                     start=(k == 0), stop=(k == 7))
                nc.vector.tensor_copy(sum_sb[:, dsl], pss[:])
                psq = ps_st.tile([1, 512], F32, tag="pstat", bufs=2)
                for k in range(8):
                    xq = p1sq.tile([128, 512], BF16, tag="xq", bufs=2)
                    nc.scalar.activation(xq[:], xt[k][:, sl], AF.Square)
                    nc.tensor.matmul(psq[:], ones_t[:], xq[:],
                                     start=(k == 0), stop=(k == 7))
                nc.vector.tensor_copy(sq_sb[:, dsl], psq[:])
            m_neg = p1.tile([1, L], F32, tag="rows", bufs=3)
            nc.vector.tensor_scalar_mul(m_neg[:], sum_sb[:], -1.0 / E)
            nc.vector.tensor_copy(negs[b][:], m_neg[:])
            msq = p1.tile([1, L], F32, tag="rows", bufs=3)
            nc.vector.tensor_tensor(msq[:], m_neg[:], m_neg[:], OP.mult)
            var = p1.tile([1, L], F32, tag="rows", bufs=3)
            nc.vector.scalar_tensor_tensor(var[:], sq_sb[:], 1.0 / E,
                                           msq[:], OP.mult, OP.subtract)
            lnv = p1.tile([1, L], F32, tag="rows", bufs=3)
            nc.scalar.activation(lnv[:], var[:], AF.Ln, bias=eps_t[0:1, :])
            r_sb = p1.tile([1, L], F32, tag="rows", bufs=3)
            nc.scalar.activation(r_sb[:], lnv[:], AF.Exp, scale=-0.5)
            for hh in range(2):
                prb = ps_rb.tile([128, 512], F32, tag="prb")
                nc.tensor.matmul(prb[:], ones_row_f[:],
                                 r_sb[:, hh * 512:(hh + 1) * 512],
                                 start=True, stop=True)
                nc.vector.tensor_copy(r_reps[b][:, hh * 512:(hh + 1) * 512],
                                      prb[:])

            # ---- in_proj (xm tiles first, z tiles after conv) ----
            def inproj_tile(mt, ch):
                col = ch * 512
                sl = slice(b * L + col, b * L + col + 512)
                ps = ps_in.tile([128, 512], F32, tag="ps", bufs=2)
                for k in range(8):
                    nc.tensor.matmul(
                        ps[:],
                        win_t[:, k * 512 + mt * 128:k * 512 + (mt + 1) * 128],
                        xt[k][:, sl], start=(k == 0), stop=False)
                nc.tensor.matmul(ps[:], sw_t[:, mt * 128:(mt + 1) * 128],
                                 negs[b][:, col:col + 512],
                                 start=False, stop=True)
                if mt < 2:
                    dst = xmp[mt][b][:, 3 + col:3 + col + 512]
                else:
                    dst = zt[mt - 2][b][:, col:col + 512]
                if ln1b_nonzero:
                    tmp = p1sq.tile([128, 512], F32, tag="eptmp")
                    nc.vector.tensor_tensor(tmp[:], ps[:],
                                            r_reps[b][:, col:col + 512],
                                            OP.mult)
                    nc.scalar.activation(dst, tmp[:], AF.Identity,
                                         bias=sbin_t[:, mt:mt + 1])
                else:
                    nc.vector.tensor_tensor(dst, ps[:],
                                            r_reps[b][:, col:col + 512],
                                            OP.mult)

            for mt in range(2):
                for ch in range(2):
                    inproj_tile(mt, ch)

            # ---- conv + silu ----
            for d in range(2):
                acc0 = cvp.tile([128, L], BF16, tag="acc", bufs=2)
                nc.vector.tensor_scalar_mul(acc0[:], xmp[d][b][:, 0:L],
                                            convw_t[:, d * KC:d * KC + 1])
                for k in (1, 2, 3):
                    acc1 = cvp.tile([128, L], BF16, tag="acc", bufs=2)
                    nc.vector.scalar_tensor_tensor(
                        acc1[:], xmp[d][b][:, k:k + L],
                        convw_t[:, d * KC + k:d * KC + k + 1],
                        acc0[:], OP.mult, OP.add)
                    acc0 = acc1
                nc.scalar.activation(xs[d][b][:], acc0[:], AF.Silu,
                                     bias=convb_t[:, d:d + 1])

            # z tiles + silu(z)
            for mt in range(2, 4):
                for ch in range(2):
                    inproj_tile(mt, ch)
            for d in range(2):
                nc.scalar.activation(zs[d][b][:], zt[d][b][:], AF.Silu)

            # ---- x_proj partial + AllReduce ----
            xdblp = xpp.tile([96, L], F32, tag="xdblp", bufs=1)
            for ch in range(2):
                col = ch * 512
                psx = ps_xp.tile([96, 512], F32, tag="psx")
                for k in range(2):
                    nc.tensor.matmul(psx[:], xpw_t[:, k * 96:(k + 1) * 96],
                                     xs[k][b][:, col:col + 512],
                                     start=(k == 0), stop=(k == 1))
                nc.vector.tensor_copy(xdblp[:, col:col + 512], psx[:])
            nc.sync.dma_start(ar_ins[b][:], xdblp[:])
            nc.gpsimd.collective_compute("AllReduce", OP.add,
                                         ins=[ar_ins[b][:]],
                                         outs=[ar_outs[b][:]],
                                         replica_groups=RG)

        def phase_dt(b):
            """dt softplus + dtx for half b (after AR(b))."""
            dtr_b = dts.tile([64, L], BF16, tag="dtrb", bufs=1)
            nc.gpsimd.dma_start(dtr_b[:], ar_outs[b][0:64, :])
            nc.gpsimd.dma_start(bc_bfs[b][:], ar_outs[b][64:96, :])
            for mt in range(2):
                dte_t = dts.tile([128, L], BF16, tag="dte")
                for ch in range(2):
                    col = ch * 512
                    psd = ps_in.tile([128, 512], F32, tag="ps", bufs=2)
                    nc.tensor.matmul(psd[:],
                                     dtw_t[:, mt * 128:(mt + 1) * 128],
                                     dtr_b[:, col:col + 512],
                                     start=True, stop=True)
                    nc.scalar.activation(dte_t[:, col:col + 512],
                                         psd[:], AF.Exp,
                                         bias=dtb_t[:, mt:mt + 1])
                nc.scalar.activation(dt_t[mt][b][:], dte_t[:],
                                     AF.Ln, bias=1.0)
                nc.vector.tensor_tensor(dtx[mt][b][:], dt_t[mt][b][:],
                                        xs[mt][b][:], OP.mult)

        psy_cur = [None]

        def scan_group(d, b, g):
            if g == 0:
                psy_cur[0] = ps_y.tile([128, L], F32, tag="psy", name="psy")
            psy = psy_cur[0]
            a_t = pa.tile([128, G, L], BF16, tag="a")
            for j in range(G):
                n = g * G + j
                nc.scalar.activation(
                    a_t[:, j, :], dt_t[d][b][:], AF.Exp,
                    scale=asc_t[:, d * NST + n:d * NST + n + 1])
            nc.vector.memset(a_t[:, :, 0:1], 0.0)
            brep = pr.tile([128, G, L], BF16, tag="bcr")
            nc.scalar.dma_start(brep[:],
                                _rep0(bc_bfs[b][g * G:(g + 1) * G, :]))
            bx = pbh.tile([128, G, L], BF16, tag="bxhc")
            dslice = dtx[d][b][:]
            dxb = bass.AP(dslice.tensor, dslice.offset,
                          [list(dslice.ap[0]), [0, G], [1, L]])
            nc.vector.tensor_tensor(bx[:], dxb, brep[:], OP.mult)
            h_t = pbh.tile([128, G, L], BF16, tag="bxhc")
            nc.vector.tensor_tensor_scan(
                h_t[:].rearrange("p a b -> p (a b)"),
                a_t[:].rearrange("p a b -> p (a b)"),
                bx[:].rearrange("p a b -> p (a b)"),
                0.0, OP.mult, OP.add)
            crep = pr.tile([128, G, L], BF16, tag="bcr")
            nc.scalar.dma_start(crep[:],
                                _rep0(bc_bfs[b][16 + g * G:16 + (g + 1) * G, :]))
            hc = pbh.tile([128, G, L], BF16, tag="bxhc")
            nc.vector.tensor_tensor(hc[:], h_t[:], crep[:], OP.mult)
            for j in range(G):
                for hh in range(2):
                    nc.tensor.matmul(
                        psy[:, hh * 512:(hh + 1) * 512], ident_t[:],
                        hc[:, j, hh * 512:(hh + 1) * 512],
                        start=(g == 0 and j == 0),
                        stop=(g == NG - 1 and j == G - 1))

        def scan_fin(d, b):
            """y2/y3 + A2A staging + launch A2A(d, b)."""
            psy = psy_cur[0]
            y2 = py.tile([128, L], BF16, tag="y2")
            nc.vector.scalar_tensor_tensor(y2[:], xs[d][b][:],
                                           dvec_t[:, d:d + 1], psy[:],
                                           OP.mult, OP.add)
            y3 = py.tile([128, L], BF16, tag="y2")
            nc.vector.tensor_tensor(y3[:], y2[:], zs[d][b][:], OP.mult)
            # single strided DMA stages all 8 destination rows
            nc.sync.dma_start(
                a2a_ins[(d, b)].rearrange("c (p q) -> p c q", p=128),
                y3[:].rearrange("p (c q) -> p c q", c=NC))
            nc.gpsimd.collective_compute("AllToAll", OP.bypass,
                                         ins=[a2a_ins[(d, b)][:]],
                                         outs=[a2a_outs[(d, b)][:]],
                                         replica_groups=RG)

        # ================= emission: head =================
        phase_A(0)
        phase_A(1)      # tensor/vector fill the AR(b0) wait
        phase_dt(0)

        # ================= emission: mid1 (scan b0) =================
        scan_group(0, 0, 0)
        scan_group(0, 0, 1)
        scan_group(0, 0, 2)
        scan_group(0, 0, 3)
        scan_fin(0, 0)
        phase_dt(1)
        scan_group(1, 0, 0)
        scan_group(1, 0, 1)
        scan_group(1, 0, 2)
        scan_group(1, 0, 3)
        scan_fin(1, 0)

        _astk.close()   # frees xt/win pools + phase-A PSUM
        _iostk.close()  # frees xmp/zt rings
        _mb0stk.close()  # frees b0 scan activations

        # MLP-side pools + weight tiles (wo resident; wfc/wpj streamed)
        mlpp = _stk.enter_context(tc.tile_pool(name="mlp", bufs=1, side="right"))
        wo_t = mlpp.tile([128, 16 * 1024], BF16, tag="wo")
        nc.sync.dma_start(wo_t[:], wo[:])

        opool = _stk.enter_context(tc.tile_pool(name="opool", bufs=1))
        wfcp = _stk.enter_context(tc.tile_pool(name="wfcp", bufs=2, side="right"))
        wpjp = _stk.enter_context(tc.tile_pool(name="wpjp", bufs=2, side="right"))

        def phase_O(b, interleave=None):
            """out_proj + LN2 + MLP for half b's 128 tokens.

            interleave: dict step-name -> fn emitting scan work between
            tensor-heavy steps.
            """
            steps = dict(interleave or {})

            def run(tag):
                if tag in steps:
                    steps.pop(tag)()

            with tc.tile_pool(name="ps_op", bufs=1, space="PSUM") as ps_op, \
                 tc.tile_pool(name="yfp", bufs=1) as yfp:
                r1T_ps = ps_op.tile([128, E], F32, tag="r1T")
                yf = []
                for d in range(2):
                    t = yfp.tile([128, NC * TOKB], BF16, tag=f"yf{d}",
                                 name=f"yf{d}_{b}")
                    nc.sync.dma_start(
                        t[:].rearrange("p (i q) -> p i q", i=NC),
                        a2a_outs[(d, b)].rearrange("i (p q) -> p i q", p=128))
                    yf.append(t)
                run("yf")
                for d in range(2):
                    for i in range(NC):
                        ci = i * 2 + d
                        for hh in range(2):
                            nc.tensor.matmul(
                                r1T_ps[:, hh * 512:(hh + 1) * 512],
                                yf[d][:, i * TOKB:(i + 1) * TOKB],
                                wo_t[:, ci * 1024 + hh * 512:
                                     ci * 1024 + (hh + 1) * 512],
                                start=(d == 0 and i == 0),
                                stop=(d == 1 and i == NC - 1),
                                skip_group_check=True)
                run("op")
                r1fT = opool.tile([128, E], F32, tag="r1fT", bufs=1,
                                  name=f"r1fT{b}")
                nc.vector.tensor_tensor(r1fT[:], r1T_ps[:], xresT_t[b][:],
                                        OP.add)

            # LN2 stats on scalar engine (per-partition = per-token)
            scr = opool.tile([128, E], F32, tag="scr", bufs=1)
            s1 = opool.tile([128, 1], F32, tag="s1", bufs=2)
            s2 = opool.tile([128, 1], F32, tag="s2", bufs=2)
            nc.scalar.activation(scr[:], r1fT[:], AF.Identity, accum_out=s1[:])
            nc.scalar.activation(scr[:], r1fT[:], AF.Square, accum_out=s2[:])
            m_neg = opool.tile([128, 1], F32, tag="mneg", bufs=2)
            nc.vector.tensor_scalar_mul(m_neg[:], s1[:], -1.0 / E)
            msq = opool.tile([128, 1], F32, tag="msq", bufs=2)
            nc.vector.tensor_tensor(msq[:], m_neg[:], m_neg[:], OP.mult)
            var = opool.tile([128, 1], F32, tag="var", bufs=2)
            nc.vector.scalar_tensor_tensor(var[:], s2[:], 1.0 / E, msq[:],
                                           OP.mult, OP.subtract)
            lnv = opool.tile([128, 1], F32, tag="lnv", bufs=2)
            nc.scalar.activation(lnv[:], var[:], AF.Ln, bias=eps_t[:])
            rstd = opool.tile([128, 1], F32, tag="rstd", bufs=2)
            nc.scalar.activation(rstd[:], lnv[:], AF.Exp, scale=-0.5)
            nmb = opool.tile([128, 1], F32, tag="nmb", bufs=2)
            nc.vector.tensor_tensor(nmb[:], m_neg[:], rstd[:], OP.mult)
            r1nT = opool.tile([128, E], BF16, tag="r1nT", bufs=1,
                              name=f"r1nT{b}")
            nc.scalar.activation(r1nT[:], r1fT[:], AF.Identity,
                                 bias=nmb[:], scale=rstd[:])
            # transpose r1nT chunks -> [e, tok] moving operand for fc
            r1tt = opool.tile([128, E], BF16, tag="r1tt", bufs=1,
                              name=f"r1tt{b}")
            for k in range(8):
                nc.scalar.dma_start_transpose(r1tt[:, k * 128:(k + 1) * 128],
                                              r1nT[:, k * 128:(k + 1) * 128])
            run("r1")

            # fc (hid on partitions): stationary wfc chunks streamed in 4
            # slabs of [128, 8(k) x 1024(hid)]; gelu bias per-partition.
            h1_t = opool.tile([128, HID], BF16, tag="h1", bufs=1,
                              name=f"h1_{b}")
            with tc.tile_pool(name="ps_fc", bufs=4, space="PSUM") as ps_fc:
                for mtg in range(8):
                    wslab = wfcp.tile([128, 8 * 512], BF16, tag="wfc",
                                      name=f"wfc{b}_{mtg}")
                    nc.scalar.dma_start(
                        wslab[:].rearrange("p (k m) -> p k m", k=8),
                        bass.AP(wfc.tensor, wfc.offset + mtg * 512,
                                [list(wfc.ap[0]), [HID, 8], [1, 512]]))
                    for mtl in range(4):
                        mt = mtg * 4 + mtl
                        psf = ps_fc.tile([128, TOKB], F32, tag="psf", bufs=4)
                        for k in range(8):
                            nc.tensor.matmul(
                                psf[:],
                                wslab[:, k * 512 + mtl * 128:
                                      k * 512 + (mtl + 1) * 128],
                                r1tt[:, k * 128:(k + 1) * 128],
                                start=(k == 0), stop=(k == 7))
                        nc.scalar.activation(h1_t[:, mt * 128:(mt + 1) * 128],
                                             psf[:], AF.Gelu,
                                             bias=sbfc_t[:, mt:mt + 1])
                    if mtg % 2 == 1:
                        run(f"fc{mtg // 2}")

            # proj: stationary = h1 tiles; moving = streamed wpj slabs
            with tc.tile_pool(name="ps_pj", bufs=1, space="PSUM") as ps_pj:
                psp = ps_pj.tile([128, E], F32, tag="psp")
                for jg in range(8):
                    wpj_g = wpjp.tile([128, 4 * E], BF16, tag="wpjg",
                                      name=f"wpj{b}_{jg}")
                    nc.scalar.dma_start(wpj_g[:],
                                        wpj[:, jg * 4 * E:(jg + 1) * 4 * E])
                    for jj in range(4):
                        j = jg * 4 + jj
                        for hh in range(2):
                            nc.tensor.matmul(
                                psp[:, hh * 512:(hh + 1) * 512],
                                h1_t[:, j * 128:(j + 1) * 128],
                                wpj_g[:, jj * E + hh * 512:
                                      jj * E + (hh + 1) * 512],
                                start=(j == 0), stop=False,
                                skip_group_check=True)
                    if jg % 2 == 1:
                        run(f"pj{jg // 2}")
                for hh in range(2):
                    nc.tensor.matmul(psp[:, hh * 512:(hh + 1) * 512],
                                     ones_row[:],
                                     pjb_t[:, hh * 512:(hh + 1) * 512],
                                     start=False, stop=True,
                                     skip_group_check=True)
                ot = opool.tile([128, E], F32, tag="ot", bufs=1)
                nc.vector.tensor_tensor(ot[:], psp[:], r1fT[:], OP.add)
                nc.sync.dma_start(outTT[b * TOKB:(b + 1) * TOKB, :], ot[:])
            # any steps not yet consumed
            for tag in list(steps):
                steps.pop(tag)()

        # ================= emission: mid2 (scan b1 || O(b0)) =================
        sg = scan_group
        phase_O(0, interleave={
            "yf": lambda: sg(0, 1, 0),
            "op": lambda: sg(0, 1, 1),
            "r1": lambda: sg(0, 1, 2),
            "fc0": lambda: (sg(0, 1, 3), scan_fin(0, 1)),
            "fc1": lambda: sg(1, 1, 0),
            "fc2": lambda: sg(1, 1, 1),
            "fc3": lambda: sg(1, 1, 2),
            "pj1": lambda: (sg(1, 1, 3), scan_fin(1, 1)),
        })

        # ================= emission: tail (O(b1)) =================
        phase_O(1)

    nc.compile()
    _BUILD_CACHE[key] = nc
    return nc


def _prep_inputs(inputs):
    """Host-side sharding/packing. Returns list of per-core input dicts."""
    f32 = np.float32
    x = np.asarray(inputs["x"], f32)
    ln1_w = np.asarray(inputs["ln1_w"], f32)
    ln1_b = np.asarray(inputs["ln1_b"], f32)
    in_proj_w = np.asarray(inputs["in_proj_w"], f32)
    conv_w = np.asarray(inputs["conv_w"], f32)
    conv_b = np.asarray(inputs["conv_b"], f32)
    x_proj_w = np.asarray(inputs["x_proj_w"], f32)
    dt_proj_w = np.asarray(inputs["dt_proj_w"], f32)
    dt_proj_b = np.asarray(inputs["dt_proj_b"], f32)
    A_log = np.asarray(inputs["A_log"], f32)
    D = np.asarray(inputs["D"], f32)
    out_proj_w = np.asarray(inputs["out_proj_w"], f32)
    ln2_w = np.asarray(inputs["ln2_w"], f32)
    ln2_b = np.asarray(inputs["ln2_b"], f32)
    fc_w = np.asarray(inputs["fc_w"], f32)
    fc_b = np.asarray(inputs["fc_b"], f32)
    proj_w = np.asarray(inputs["proj_w"], f32)
    proj_b = np.asarray(inputs["proj_b"], f32)

    x2d = np.ascontiguousarray(x.reshape(TOK, E))             # [TOK, E]
    xT_b = np.ascontiguousarray(x2d.T).astype(bf)             # [E, TOK]

    def pack_lhsT(lhsT):
        K, M = lhsT.shape
        nk = K // 128
        return np.ascontiguousarray(
            lhsT.reshape(nk, 128, M).transpose(1, 0, 2).reshape(128, nk * M)
        ).astype(bf)

    Wp = in_proj_w * ln1_w[None, :]
    sb_full = in_proj_w @ ln1_b
    ln1b_nonzero = bool(np.any(sb_full != 0.0))

    Wfc = fc_w * ln2_w[None, :]
    sbfc_full = fc_w @ ln2_b + fc_b
    wfc_pack = pack_lhsT(np.ascontiguousarray(Wfc.T))
    wpj_pack = pack_lhsT(np.ascontiguousarray(proj_w.T))
    woT = np.ascontiguousarray(out_proj_w.T)                  # [DIN, E]
    wo_pack = np.ascontiguousarray(np.hstack(
        [woT[ci * 128:(ci + 1) * 128, :] for ci in range(16)])).astype(bf)
    sbfc_pack = np.ascontiguousarray(sbfc_full.reshape(32, 128).T).astype(f32)
    pjb_row = proj_b[None, :].astype(bf)

    A = -np.exp(A_log)

    per_core = []
    for c in range(NC):
        dsl = slice(c * DL, (c + 1) * DL)
        rows = np.concatenate([Wp[dsl], Wp[DIN + c * DL:DIN + (c + 1) * DL]])
        win_pack = pack_lhsT(np.ascontiguousarray(rows.T))
        sw_row = rows.sum(1)[None, :].astype(bf)
        sb_rows = np.concatenate([sb_full[dsl],
                                  sb_full[DIN + c * DL:DIN + (c + 1) * DL]])
        sb_pack = np.ascontiguousarray(sb_rows.reshape(4, 128).T).astype(f32)

        cw = conv_w[dsl, 0, :]
        convw_pack = np.ascontiguousarray(
            cw.reshape(2, 128, KC).transpose(1, 0, 2).reshape(128, 2 * KC)
        ).astype(f32)
        convb_pack = np.ascontiguousarray(
            conv_b[dsl].reshape(2, 128).T).astype(f32)

        xpw_pack = pack_lhsT(np.ascontiguousarray(x_proj_w[:, dsl].T))
        dtw_slice = np.ascontiguousarray(dt_proj_w[dsl].T).astype(bf)
        dtb_pack = np.ascontiguousarray(
            dt_proj_b[dsl].reshape(2, 128).T).astype(f32)
        asc_pack = np.ascontiguousarray(
            A[dsl].reshape(2, 128, NST).transpose(1, 0, 2).reshape(128, 2 * NST)
        ).astype(f32)
        dvec_pack = np.ascontiguousarray(D[dsl].reshape(2, 128).T).astype(f32)

        xresT_slice = np.ascontiguousarray(np.concatenate(
            [x2d[c * TOKB:(c + 1) * TOKB, :],
             x2d[L + c * TOKB:L + (c + 1) * TOKB, :]]))       # [256, E]

        per_core.append({
            "xT": xT_b, "win": win_pack, "sw_in": sw_row, "sb_in": sb_pack,
            "convw": convw_pack, "convb": convb_pack, "xpw": xpw_pack,
            "dtw": dtw_slice, "dtb": dtb_pack, "a_sc": asc_pack,
            "dvec": dvec_pack, "wo": wo_pack, "xresT": xresT_slice,
            "wfc": wfc_pack, "sbfc": sbfc_pack,
            "wpj": wpj_pack, "pjb_row": pjb_row,
            "ones128": np.ones((128, 1), bf),
            "ident": np.eye(128, dtype=bf),
        })
    return per_core, ln1b_nonzero


def kernel(**inputs):
    per_core, ln1b_nonzero = _prep_inputs(inputs)
    nc = _build(ln1b_nonzero)
    trace = bool(int(os.environ.get("BASSK_TRACE", "0")))
    try:
        res = run_bass_kernel_spmd(nc, per_core, core_ids=list(range(NC)),
                                   trace=trace)
    except Exception:
        # transient device hiccups (e.g. NRT exec-unit errors) clear on retry
        res = run_bass_kernel_spmd(nc, per_core, core_ids=list(range(NC)),
                                   trace=trace)
    kernel.last_results = res
    out2d = np.empty((TOK, E), np.float32)
    for c in range(NC):
        r = res.results[c]["outTT"]
        out2d[c * TOKB:(c + 1) * TOKB] = r[:TOKB]
        out2d[L + c * TOKB:L + (c + 1) * TOKB] = r[TOKB:]
    return out2d.reshape(B, L, E).astype(np.float32)


# revision 18
# speedup vs baseline: 1.1183x; 1.0121x over previous
"""Mamba block (dense_transformer nn_Block) on 8 Trainium2 NeuronCores.

Batch-half pipelined schedule. d_inner sharded 8-way (256 ch/core) for
in_proj/conv/scan; x_proj partials AllReduced per batch half; the scan output
is re-sharded to tokens by per-(d,half) AllToAlls; out_proj runs in a
token-transposed layout (tokens on partitions) so LN2 and its application are
pure scalar-engine work; fc consumes the transposed-back activations with
streamed weights and its output tiles feed proj directly as the stationary
operand.  The vector-engine scan of half b overlaps the tensor engine's
out_proj+MLP of half b-1.
"""
import os
import numpy as np
import ml_dtypes

import concourse.bass as bass
import concourse.bacc as bacc
import concourse.mybir as mybir
import concourse.tile as tile
from contextlib import ExitStack
from concourse.bass_utils import run_bass_kernel_spmd

BF16 = mybir.dt.bfloat16
F32 = mybir.dt.float32
AF = mybir.ActivationFunctionType
OP = mybir.AluOpType
bf = ml_dtypes.bfloat16

B, L, E = 2, 1024, 1024
DIN, NST, RDT, KC = 2 * E, 16, 64, 4
EPS = 1e-5
NC = 8
DL = DIN // NC          # 256 channels per core
TOK = B * L             # 2048
TOKB = 128              # tokens per core per batch half
HID = 4 * E             # 4096
G = 4                   # states per scan instruction
NG = NST // G

_BUILD_CACHE = {}


def _rep0(src_ap, parts=128):
    """Partition-broadcast: prepend a [0, parts] dim to an AP's pattern."""
    return bass.AP(src_ap.tensor, src_ap.offset,
                   [[0, parts]] + [list(p) for p in src_ap.ap])


def _build(ln1b_nonzero):
    key = (ln1b_nonzero,)
    if key in _BUILD_CACHE:
        return _BUILD_CACHE[key]

    nc = bacc.Bacc("TRN2", target_bir_lowering=False, debug=False, num_devices=NC)

    def din(name, shape, dt=BF16):
        return nc.dram_tensor(name, shape, dt, kind="ExternalInput").ap()

    xT = din("xT", [E, TOK])
    win = din("win", [128, 8 * 512])
    sw_in = din("sw_in", [1, 512])
    sb_in = din("sb_in", [128, 4], F32)
    convw = din("convw", [128, 2 * KC], F32)
    convb = din("convb", [128, 2], F32)
    xpw = din("xpw", [128, 2 * 96])
    dtw = din("dtw", [64, 256])
    dtb = din("dtb", [128, 2], F32)
    a_sc = din("a_sc", [128, 2 * NST], F32)
    dvec = din("dvec", [128, 2], F32)
    wo = din("wo", [128, 16 * 1024])
    xresT = din("xresT", [2 * TOKB, E], F32)
    wfc = din("wfc", [128, 8 * HID])
    sbfc = din("sbfc", [128, 32], F32)
    wpj = din("wpj", [128, 32 * E])
    pjb_row = din("pjb_row", [1, E])
    ones128 = din("ones128", [128, 1])
    ident = din("ident", [128, 128])

    outTT = nc.dram_tensor("outTT", [2 * TOKB, E], F32, kind="ExternalOutput").ap()

    cc_dummy_in = nc.dram_tensor("cc_dummy_in", [1, 16], F32)
    cc_dummy_out = nc.dram_tensor("cc_dummy_out", [1, 16], F32, addr_space="Shared")
    ar_ins = [nc.dram_tensor(f"ar_in{b}", [96, L], F32) for b in range(2)]
    ar_outs = [nc.dram_tensor(f"ar_out{b}", [96, L], F32, addr_space="Shared")
               for b in range(2)]
    bc_bfs = [nc.dram_tensor(f"bc_bf{b}", [32, L], BF16) for b in range(2)]
    a2a_ins = [nc.dram_tensor(f"a2a_in{b}", [NC, 2 * 128 * TOKB], BF16)
               for b in range(2)]
    a2a_outs = [nc.dram_tensor(f"a2a_out{b}", [NC, 2 * 128 * TOKB], BF16)
                for b in range(2)]
    RG = [list(range(NC))]

    with tile.TileContext(nc) as tc, ExitStack() as _stk:
        # warm the collective stream early (absorbs ~80us barrier + delay)
        nc.gpsimd.collective_compute("AllReduce", OP.add, ins=[cc_dummy_in[:]],
                                     outs=[cc_dummy_out[:]], replica_groups=RG)

        cpool = _stk.enter_context(tc.tile_pool(name="consts", bufs=1))
        ones_t = cpool.tile([128, 1], BF16, tag="ones")
        nc.sync.dma_start(ones_t[:], ones128[:])
        ident_t = cpool.tile([128, 128], BF16, tag="ident")
        nc.sync.dma_start(ident_t[:], ident[:])
        ones_row = cpool.tile([1, 128], BF16, tag="onesrow")
        nc.sync.dma_start(ones_row[:], ones128[:].rearrange("p q -> q p"))
        ones_row_f = cpool.tile([1, 128], F32, tag="onesrowf")
        nc.vector.tensor_copy(ones_row_f[:], ones_row[:])
        sw_t = cpool.tile([1, 512], BF16, tag="sw")
        nc.sync.dma_start(sw_t[:], sw_in[:])
        convw_t = cpool.tile([128, 2 * KC], F32, tag="convw")
        nc.sync.dma_start(convw_t[:], convw[:])
        convb_t = cpool.tile([128, 2], F32, tag="convb")
        nc.sync.dma_start(convb_t[:], convb[:])
        xpw_t = cpool.tile([128, 2 * 96], BF16, tag="xpw")
        nc.sync.dma_start(xpw_t[:], xpw[:])
        dtw_t = cpool.tile([64, 256], BF16, tag="dtw")
        nc.sync.dma_start(dtw_t[:], dtw[:])
        dtb_t = cpool.tile([128, 2], F32, tag="dtb")
        nc.sync.dma_start(dtb_t[:], dtb[:])
        asc_t = cpool.tile([128, 2 * NST], F32, tag="asc")
        nc.sync.dma_start(asc_t[:], a_sc[:])
        dvec_t = cpool.tile([128, 2], F32, tag="dvec")
        nc.sync.dma_start(dvec_t[:], dvec[:])
        sbfc_t = cpool.tile([128, 32], F32, tag="sbfc")
        nc.sync.dma_start(sbfc_t[:], sbfc[:])
        pjb_t = cpool.tile([1, E], BF16, tag="pjb")
        nc.sync.dma_start(pjb_t[:], pjb_row[:])
        eps_t = cpool.tile([128, 1], F32, tag="eps")
        nc.vector.memset(eps_t[:], EPS)
        sbin_t = cpool.tile([128, 4], F32, tag="sbin")
        if ln1b_nonzero:
            nc.sync.dma_start(sbin_t[:], sb_in[:])
        xresT_t = [cpool.tile([TOKB, E], F32, tag=f"xresT{b}",
                              name=f"xresT_t{b}") for b in range(2)]
        for b in range(2):
            nc.sync.dma_start(xresT_t[b][:], xresT[b * TOKB:(b + 1) * TOKB, :])

        # ---- long-lived pools (to program end) ----
        xmp = [[None, None], [None, None]]
        zt = [[None, None], [None, None]]
        mbp1 = _stk.enter_context(tc.tile_pool(name="mamba1", bufs=1))
        pa = _stk.enter_context(tc.tile_pool(name="scan_a", bufs=3))
        pbh = _stk.enter_context(tc.tile_pool(name="scan_bh", bufs=4))
        pr = _stk.enter_context(tc.tile_pool(name="scan_r", bufs=2))
        py = _stk.enter_context(tc.tile_pool(name="scan_y", bufs=2))
        ps_y = _stk.enter_context(tc.tile_pool(name="ps_y", bufs=1, space="PSUM"))
        # ---- short-lived pools (close at mid2 start) ----
        _mb0stk = ExitStack()
        mbp0 = _mb0stk.enter_context(tc.tile_pool(name="mamba0", bufs=1))
        _iostk = ExitStack()
        iop = _iostk.enter_context(tc.tile_pool(name="mamba_io", bufs=1))
        mbp = [mbp0, mbp1]
        zs = [[mbp[b].tile([128, L], BF16, tag=f"zs{d}", name=f"zs{d}{b}")
               for b in range(2)] for d in range(2)]
        xs = [[mbp[b].tile([128, L], BF16, tag=f"xs{d}", name=f"xs{d}{b}")
               for b in range(2)] for d in range(2)]
        dt_t = [[mbp[b].tile([128, L], BF16, tag=f"dt{d}", name=f"dt{d}{b}")
                 for b in range(2)] for d in range(2)]
        dtx = [[mbp[b].tile([128, L], BF16, tag=f"dtx{d}", name=f"dtx{d}{b}")
                for b in range(2)] for d in range(2)]

        # ---- phase-A pools (head + mid1) ----
        _astk = ExitStack()
        p1 = _astk.enter_context(tc.tile_pool(name="ph1", bufs=1))
        p1sq = _astk.enter_context(tc.tile_pool(name="ph1sq", bufs=3))
        ps_st = _astk.enter_context(tc.tile_pool(name="ps_st", bufs=1, space="PSUM"))
        ps_in = _astk.enter_context(tc.tile_pool(name="ps_in", bufs=2, space="PSUM"))
        ps_rb = _astk.enter_context(tc.tile_pool(name="ps_rb", bufs=1, space="PSUM"))
        cvp = _astk.enter_context(tc.tile_pool(name="conv", bufs=2))
        xpp = _astk.enter_context(tc.tile_pool(name="xp", bufs=2))
        ps_xp = _astk.enter_context(tc.tile_pool(name="ps_xp", bufs=1, space="PSUM"))
        dts = _astk.enter_context(tc.tile_pool(name="dts", bufs=2))

        xt = [p1.tile([128, TOK], BF16, tag=f"xt{k}", name=f"xt{k}")
              for k in range(8)]
        for k in range(8):
            nc.sync.dma_start(xt[k][:], xT[k * 128:(k + 1) * 128, :])
        win_t = p1.tile([128, 8 * 512], BF16, tag="win")
        nc.sync.dma_start(win_t[:], win[:])
        negs = [p1.tile([1, L], BF16, tag=f"negm{b}", name=f"negm{b}")
                for b in range(2)]
        r_reps = [p1.tile([128, L], BF16, tag="r_rep", name=f"r_rep{b}")
                  for b in range(2)]

        def phase_A1(b):
            """LN1 stats, in_proj xm tiles, conv+silu for half b."""
            for d in range(2):
                xmp[d][b] = iop.tile([128, 3 + L], BF16, tag=f"xmp{d}",
                                     name=f"xmp{d}{b}")
                nc.vector.memset(xmp[d][b][:, 0:3], 0.0)
                zt[d][b] = iop.tile([128, L], BF16, tag=f"z{d}",
                                    name=f"zt{d}{b}")
            # ---- LN1 stats ----
            sum_sb = p1.tile([1, L], F32, tag="rows", bufs=3)
            sq_sb = p1.tile([1, L], F32, tag="rows", bufs=3)
            for ch in range(2):
                sl = slice(b * L + ch * 512, b * L + (ch + 1) * 512)
                dsl = slice(ch * 512, (ch + 1) * 512)
                pss = ps_st.tile([1, 512], F32, tag="pstat", bufs=2)
                for k in range(8):
                    nc.tensor.matmul(pss[:], ones_t[:], xt[k][:, sl],
                                     start=(k == 0), stop=(k == 7))
                nc.vector.tensor_copy(sum_sb[:, dsl], pss[:])
                psq = ps_st.tile([1, 512], F32, tag="pstat", bufs=2)
                for k in range(8):
                    xq = p1sq.tile([128, 512], BF16, tag="xq", bufs=2)
                    nc.scalar.activation(xq[:], xt[k][:, sl], AF.Square)
                    nc.tensor.matmul(psq[:], ones_t[:], xq[:],
                                     start=(k == 0), stop=(k == 7))
                nc.vector.tensor_copy(sq_sb[:, dsl], psq[:])
            m_neg = p1.tile([1, L], F32, tag="rows", bufs=3)
            nc.vector.tensor_scalar_mul(m_neg[:], sum_sb[:], -1.0 / E)
            nc.vector.tensor_copy(negs[b][:], m_neg[:])
            msq = p1.tile([1, L], F32, tag="rows", bufs=3)
            nc.vector.tensor_tensor(msq[:], m_neg[:], m_neg[:], OP.mult)
            var = p1.tile([1, L], F32, tag="rows", bufs=3)
            nc.vector.scalar_tensor_tensor(var[:], sq_sb[:], 1.0 / E,
                                           msq[:], OP.mult, OP.subtract)
            lnv = p1.tile([1, L], F32, tag="rows", bufs=3)
            nc.scalar.activation(lnv[:], var[:], AF.Ln, bias=eps_t[0:1, :])
            r_sb = p1.tile([1, L], F32, tag="rows", bufs=3)
            nc.scalar.activation(r_sb[:], lnv[:], AF.Exp, scale=-0.5)
            for hh in range(2):
                prb = ps_rb.tile([128, 512], F32, tag="prb")
                nc.tensor.matmul(prb[:], ones_row_f[:],
                                 r_sb[:, hh * 512:(hh + 1) * 512],
                                 start=True, stop=True)
                nc.vector.tensor_copy(r_reps[b][:, hh * 512:(hh + 1) * 512],
                                      prb[:])

            # ---- in_proj (xm tiles first, z tiles after conv) ----
            def inproj_tile(mt, ch):
                col = ch * 512
                sl = slice(b * L + col, b * L + col + 512)
                ps = ps_in.tile([128, 512], F32, tag="ps", bufs=2)
                for k in range(8):
                    nc.tensor.matmul(
                        ps[:],
                        win_t[:, k * 512 + mt * 128:k * 512 + (mt + 1) * 128],
                        xt[k][:, sl], start=(k == 0), stop=False)
                nc.tensor.matmul(ps[:], sw_t[:, mt * 128:(mt + 1) * 128],
                                 negs[b][:, col:col + 512],
                                 start=False, stop=True)
                if mt < 2:
                    dst = xmp[mt][b][:, 3 + col:3 + col + 512]
                else:
                    dst = zt[mt - 2][b][:, col:col + 512]
                if ln1b_nonzero:
                    tmp = p1sq.tile([128, 512], F32, tag="eptmp")
                    nc.vector.tensor_tensor(tmp[:], ps[:],
                                            r_reps[b][:, col:col + 512],
                                            OP.mult)
                    nc.scalar.activation(dst, tmp[:], AF.Identity,
                                         bias=sbin_t[:, mt:mt + 1])
                else:
                    nc.vector.tensor_tensor(dst, ps[:],
                                            r_reps[b][:, col:col + 512],
                                            OP.mult)

            for mt in range(2):
                for ch in range(2):
                    inproj_tile(mt, ch)

            # ---- conv + silu ----
            for d in range(2):
                acc0 = cvp.tile([128, L], BF16, tag="acc", bufs=2)
                nc.vector.tensor_scalar_mul(acc0[:], xmp[d][b][:, 0:L],
                                            convw_t[:, d * KC:d * KC + 1])
                for k in (1, 2, 3):
                    acc1 = cvp.tile([128, L], BF16, tag="acc", bufs=2)
                    nc.vector.scalar_tensor_tensor(
                        acc1[:], xmp[d][b][:, k:k + L],
                        convw_t[:, d * KC + k:d * KC + k + 1],
                        acc0[:], OP.mult, OP.add)
                    acc0 = acc1
                nc.scalar.activation(xs[d][b][:], acc0[:], AF.Silu,
                                     bias=convb_t[:, d:d + 1])

            phase_A1._tail[b] = inproj_tile

        phase_A1._tail = [None, None]

        def phase_A2(b):
            """in_proj z tiles, silu(z), x_proj partial + AllReduce."""
            inproj_tile = phase_A1._tail[b]
            for mt in range(2, 4):
                for ch in range(2):
                    inproj_tile(mt, ch)
            for d in range(2):
                nc.scalar.activation(zs[d][b][:], zt[d][b][:], AF.Silu)

            # ---- x_proj partial + AllReduce ----
            xdblp = xpp.tile([96, L], F32, tag="xdblp", bufs=1)
            for ch in range(2):
                col = ch * 512
                psx = ps_xp.tile([96, 512], F32, tag="psx")
                for k in range(2):
                    nc.tensor.matmul(psx[:], xpw_t[:, k * 96:(k + 1) * 96],
                                     xs[k][b][:, col:col + 512],
                                     start=(k == 0), stop=(k == 1))
                nc.vector.tensor_copy(xdblp[:, col:col + 512], psx[:])
            nc.sync.dma_start(ar_ins[b][:], xdblp[:])
            nc.gpsimd.collective_compute("AllReduce", OP.add,
                                         ins=[ar_ins[b][:]],
                                         outs=[ar_outs[b][:]],
                                         replica_groups=RG)

        def phase_dt(b):
            """dt softplus + dtx for half b (after AR(b))."""
            dtr_b = dts.tile([64, L], BF16, tag="dtrb", bufs=1)
            nc.gpsimd.dma_start(dtr_b[:], ar_outs[b][0:64, :])
            nc.gpsimd.dma_start(bc_bfs[b][:], ar_outs[b][64:96, :])
            dtes = []
            for mt in range(2):
                dte_t = dts.tile([128, L], BF16, tag="dte")
                for ch in range(2):
                    col = ch * 512
                    psd = ps_in.tile([128, 512], F32, tag="ps", bufs=2)
                    nc.tensor.matmul(psd[:],
                                     dtw_t[:, mt * 128:(mt + 1) * 128],
                                     dtr_b[:, col:col + 512],
                                     start=True, stop=True)
                    nc.scalar.activation(dte_t[:, col:col + 512],
                                         psd[:], AF.Exp,
                                         bias=dtb_t[:, mt:mt + 1])
                dtes.append(dte_t)
            for mt in range(2):
                nc.scalar.activation(dt_t[mt][b][:], dtes[mt][:],
                                     AF.Ln, bias=1.0)
                nc.vector.tensor_tensor(dtx[mt][b][:], dt_t[mt][b][:],
                                        xs[mt][b][:], OP.mult)

        psy_cur = [None]

        def scan_group(d, b, g):
            if g == 0:
                psy_cur[0] = ps_y.tile([128, L], F32, tag="psy", name="psy")
            psy = psy_cur[0]
            a_t = pa.tile([128, G, L], BF16, tag="a")
            for j in range(G):
                n = g * G + j
                nc.scalar.activation(
                    a_t[:, j, :], dt_t[d][b][:], AF.Exp,
                    scale=asc_t[:, d * NST + n:d * NST + n + 1])
            nc.vector.memset(a_t[:, :, 0:1], 0.0)
            brep = pr.tile([128, G, L], BF16, tag="bcr")
            nc.sync.dma_start(brep[:],
                              _rep0(bc_bfs[b][g * G:(g + 1) * G, :]))
            bx = pbh.tile([128, G, L], BF16, tag="bxhc")
            dslice = dtx[d][b][:]
            dxb = bass.AP(dslice.tensor, dslice.offset,
                          [list(dslice.ap[0]), [0, G], [1, L]])
            nc.vector.tensor_tensor(bx[:], dxb, brep[:], OP.mult)
            h_t = pbh.tile([128, G, L], BF16, tag="bxhc")
            nc.vector.tensor_tensor_scan(
                h_t[:].rearrange("p a b -> p (a b)"),
                a_t[:].rearrange("p a b -> p (a b)"),
                bx[:].rearrange("p a b -> p (a b)"),
                0.0, OP.mult, OP.add)
            crep = pr.tile([128, G, L], BF16, tag="bcr")
            nc.sync.dma_start(crep[:],
                              _rep0(bc_bfs[b][16 + g * G:16 + (g + 1) * G, :]))
            hc = pbh.tile([128, G, L], BF16, tag="bxhc")
            nc.vector.tensor_tensor(hc[:], h_t[:], crep[:], OP.mult)
            for j in range(G):
                for hh in range(2):
                    nc.tensor.matmul(
                        psy[:, hh * 512:(hh + 1) * 512], ident_t[:],
                        hc[:, j, hh * 512:(hh + 1) * 512],
                        start=(g == 0 and j == 0),
                        stop=(g == NG - 1 and j == G - 1))

        def scan_fin(d, b):
            """y2/y3 + A2A staging + launch A2A(d, b)."""
            psy = psy_cur[0]
            y2 = py.tile([128, L], BF16, tag="y2")
            nc.vector.scalar_tensor_tensor(y2[:], xs[d][b][:],
                                           dvec_t[:, d:d + 1], psy[:],
                                           OP.mult, OP.add)
            y3 = py.tile([128, L], BF16, tag="y2")
            nc.vector.tensor_tensor(y3[:], y2[:], zs[d][b][:], OP.mult)
            # single strided DMA stages all 8 destination rows
            seg = a2a_ins[b][:, d * 128 * TOKB:(d + 1) * 128 * TOKB]
            nc.sync.dma_start(
                seg.rearrange("c (p q) -> p c q", p=128),
                y3[:].rearrange("p (c q) -> p c q", c=NC))
            if d == 1:
                nc.gpsimd.collective_compute("AllToAll", OP.bypass,
                                             ins=[a2a_ins[b][:]],
                                             outs=[a2a_outs[b][:]],
                                             replica_groups=RG)

        # ================= emission: head =================
        phase_A1(0)
        phase_A2(0)     # AR(b0) launches here
        phase_A1(1)     # tensor/vector fill the AR(b0) wait
        phase_dt(0)
        phase_A2(1)     # AR(b1) launches here

        # ================= emission: mid1 (scan b0) =================
        scan_group(0, 0, 0)
        scan_group(0, 0, 1)
        scan_group(0, 0, 2)
        scan_group(0, 0, 3)
        scan_fin(0, 0)
        phase_dt(1)
        scan_group(1, 0, 0)
        scan_group(1, 0, 1)
        scan_group(1, 0, 2)
        scan_group(1, 0, 3)
        scan_fin(1, 0)

        _astk.close()   # frees xt/win pools + phase-A PSUM
        _iostk.close()  # frees xmp/zt rings
        _mb0stk.close()  # frees b0 scan activations

        # MLP-side pools + weight tiles (wo resident; wfc/wpj streamed)
        mlpp = _stk.enter_context(tc.tile_pool(name="mlp", bufs=1, side="right"))
        wo_t = mlpp.tile([128, 16 * 1024], BF16, tag="wo")
        nc.sync.dma_start(wo_t[:], wo[:])

        opool = _stk.enter_context(tc.tile_pool(name="opool", bufs=1))
        wfcp = _stk.enter_context(tc.tile_pool(name="wfcp", bufs=2, side="right"))
        wpjp = _stk.enter_context(tc.tile_pool(name="wpjp", bufs=2, side="right"))

        def phase_O(b, interleave=None):
            """out_proj + LN2 + MLP for half b's 128 tokens.

            interleave: dict step-name -> fn emitting scan work between
            tensor-heavy steps.
            """
            steps = dict(interleave or {})

            def run(tag):
                if tag in steps:
                    steps.pop(tag)()

            with tc.tile_pool(name="ps_op", bufs=1, space="PSUM") as ps_op, \
                 tc.tile_pool(name="yfp", bufs=1) as yfp:
                r1T_ps = ps_op.tile([128, E], F32, tag="r1T")
                yf_t = yfp.tile([128, 2 * NC * TOKB], BF16, tag="yf",
                                name=f"yf_{b}")
                for d in range(2):
                    seg = a2a_outs[b][:, d * 128 * TOKB:(d + 1) * 128 * TOKB]
                    nc.sync.dma_start(
                        yf_t[:, d * NC * TOKB:(d + 1) * NC * TOKB]
                        .rearrange("p (i q) -> p i q", i=NC),
                        seg.rearrange("i (p q) -> p i q", p=128))
                run("yf")
                for d in range(2):
                    for i in range(NC):
                        ci = i * 2 + d
                        for hh in range(2):
                            nc.tensor.matmul(
                                r1T_ps[:, hh * 512:(hh + 1) * 512],
                                yf_t[:, (d * NC + i) * TOKB:
                                     (d * NC + i + 1) * TOKB],
                                wo_t[:, ci * 1024 + hh * 512:
                                     ci * 1024 + (hh + 1) * 512],
                                start=(d == 0 and i == 0),
                                stop=(d == 1 and i == NC - 1),
                                skip_group_check=True)
                run("op")
                r1fT = opool.tile([128, E], F32, tag="r1fT", bufs=1,
                                  name=f"r1fT{b}")
                nc.vector.tensor_tensor(r1fT[:], r1T_ps[:], xresT_t[b][:],
                                        OP.add)

            # LN2 stats on scalar engine (per-partition = per-token)
            scr = opool.tile([128, E], F32, tag="scr", bufs=1)
            s1 = opool.tile([128, 1], F32, tag="s1", bufs=2)
            s2 = opool.tile([128, 1], F32, tag="s2", bufs=2)
            nc.scalar.activation(scr[:], r1fT[:], AF.Identity, accum_out=s1[:])
            nc.scalar.activation(scr[:], r1fT[:], AF.Square, accum_out=s2[:])
            m_neg = opool.tile([128, 1], F32, tag="mneg", bufs=2)
            nc.vector.tensor_scalar_mul(m_neg[:], s1[:], -1.0 / E)
            msq = opool.tile([128, 1], F32, tag="msq", bufs=2)
            nc.vector.tensor_tensor(msq[:], m_neg[:], m_neg[:], OP.mult)
            var = opool.tile([128, 1], F32, tag="var", bufs=2)
            nc.vector.scalar_tensor_tensor(var[:], s2[:], 1.0 / E, msq[:],
                                           OP.mult, OP.subtract)
            lnv = opool.tile([128, 1], F32, tag="lnv", bufs=2)
            nc.scalar.activation(lnv[:], var[:], AF.Ln, bias=eps_t[:])
            rstd = opool.tile([128, 1], F32, tag="rstd", bufs=2)
            nc.scalar.activation(rstd[:], lnv[:], AF.Exp, scale=-0.5)
            nmb = opool.tile([128, 1], F32, tag="nmb", bufs=2)
            nc.vector.tensor_tensor(nmb[:], m_neg[:], rstd[:], OP.mult)
            r1nT = opool.tile([128, E], BF16, tag="r1nT", bufs=1,
                              name=f"r1nT{b}")
            nc.scalar.activation(r1nT[:], r1fT[:], AF.Identity,
                                 bias=nmb[:], scale=rstd[:])
            # transpose r1nT chunks -> [e, tok] moving operand for fc
            r1tt = opool.tile([128, E], BF16, tag="r1tt", bufs=1,
                              name=f"r1tt{b}")
            for k in range(8):
                nc.scalar.dma_start_transpose(r1tt[:, k * 128:(k + 1) * 128],
                                              r1nT[:, k * 128:(k + 1) * 128])
            run("r1")

            # fc (hid on partitions): stationary wfc chunks streamed in 4
            # slabs of [128, 8(k) x 1024(hid)]; gelu bias per-partition.
            h1_t = opool.tile([128, HID], BF16, tag="h1", bufs=1,
                              name=f"h1_{b}")
            with tc.tile_pool(name="ps_fc", bufs=4, space="PSUM") as ps_fc:
                for mtg in range(8):
                    wslab = wfcp.tile([128, 8 * 512], BF16, tag="wfc",
                                      name=f"wfc{b}_{mtg}")
                    nc.gpsimd.dma_start(
                        wslab[:].rearrange("p (k m) -> p k m", k=8),
                        bass.AP(wfc.tensor, wfc.offset + mtg * 512,
                                [list(wfc.ap[0]), [HID, 8], [1, 512]]))
                    for mtl in range(4):
                        mt = mtg * 4 + mtl
                        psf = ps_fc.tile([128, TOKB], F32, tag="psf", bufs=4)
                        for k in range(8):
                            nc.tensor.matmul(
                                psf[:],
                                wslab[:, k * 512 + mtl * 128:
                                      k * 512 + (mtl + 1) * 128],
                                r1tt[:, k * 128:(k + 1) * 128],
                                start=(k == 0), stop=(k == 7))
                        nc.scalar.activation(h1_t[:, mt * 128:(mt + 1) * 128],
                                             psf[:], AF.Gelu,
                                             bias=sbfc_t[:, mt:mt + 1])
                    if mtg % 2 == 1:
                        run(f"fc{mtg // 2}")

            # proj: stationary = h1 tiles; moving = streamed wpj slabs
            with tc.tile_pool(name="ps_pj", bufs=1, space="PSUM") as ps_pj:
                psp = ps_pj.tile([128, E], F32, tag="psp")
                for jg in range(8):
                    wpj_g = wpjp.tile([128, 4 * E], BF16, tag="wpjg",
                                      name=f"wpj{b}_{jg}")
                    nc.gpsimd.dma_start(
                        wpj_g[:], wpj[:, jg * 4 * E:(jg + 1) * 4 * E])
                    for jj in range(4):
                        j = jg * 4 + jj
                        for hh in range(2):
                            nc.tensor.matmul(
                                psp[:, hh * 512:(hh + 1) * 512],
                                h1_t[:, j * 128:(j + 1) * 128],
                                wpj_g[:, jj * E + hh * 512:
                                      jj * E + (hh + 1) * 512],
                                start=(j == 0), stop=False,
                                skip_group_check=True)
                    if jg % 2 == 1:
                        run(f"pj{jg // 2}")
                for hh in range(2):
                    nc.tensor.matmul(psp[:, hh * 512:(hh + 1) * 512],
                                     ones_row[:],
                                     pjb_t[:, hh * 512:(hh + 1) * 512],
                                     start=False, stop=True,
                                     skip_group_check=True)
                ot = opool.tile([128, E], F32, tag="ot", bufs=1)
                nc.vector.tensor_tensor(ot[:], psp[:], r1fT[:], OP.add)
                nc.sync.dma_start(outTT[b * TOKB:(b + 1) * TOKB, :], ot[:])
            # any steps not yet consumed
            for tag in list(steps):
                steps.pop(tag)()

        # ================= emission: mid2 (scan b1 || O(b0)) =================
        sg = scan_group
        phase_O(0, interleave={
            "yf": lambda: sg(0, 1, 0),
            "op": lambda: sg(0, 1, 1),
            "r1": lambda: sg(0, 1, 2),
            "fc0": lambda: (sg(0, 1, 3), scan_fin(0, 1)),
            "fc1": lambda: sg(1, 1, 0),
            "fc2": lambda: sg(1, 1, 1),
            "fc3": lambda: sg(1, 1, 2),
            "pj0": lambda: (sg(1, 1, 3), scan_fin(1, 1)),
        })

        # ================= emission: tail (O(b1)) =================
        phase_O(1)

    nc.compile()
    _BUILD_CACHE[key] = nc
    return nc


def _prep_inputs(inputs):
    """Host-side sharding/packing. Returns list of per-core input dicts."""
    f32 = np.float32
    x = np.asarray(inputs["x"], f32)
    ln1_w = np.asarray(inputs["ln1_w"], f32)
    ln1_b = np.asarray(inputs["ln1_b"], f32)
    in_proj_w = np.asarray(inputs["in_proj_w"], f32)
    conv_w = np.asarray(inputs["conv_w"], f32)
    conv_b = np.asarray(inputs["conv_b"], f32)
    x_proj_w = np.asarray(inputs["x_proj_w"], f32)
    dt_proj_w = np.asarray(inputs["dt_proj_w"], f32)
    dt_proj_b = np.asarray(inputs["dt_proj_b"], f32)
    A_log = np.asarray(inputs["A_log"], f32)
    D = np.asarray(inputs["D"], f32)
    out_proj_w = np.asarray(inputs["out_proj_w"], f32)
    ln2_w = np.asarray(inputs["ln2_w"], f32)
    ln2_b = np.asarray(inputs["ln2_b"], f32)
    fc_w = np.asarray(inputs["fc_w"], f32)
    fc_b = np.asarray(inputs["fc_b"], f32)
    proj_w = np.asarray(inputs["proj_w"], f32)
    proj_b = np.asarray(inputs["proj_b"], f32)

    x2d = np.ascontiguousarray(x.reshape(TOK, E))             # [TOK, E]
    xT_b = np.ascontiguousarray(x2d.T).astype(bf)             # [E, TOK]

    def pack_lhsT(lhsT):
        K, M = lhsT.shape
        nk = K // 128
        return np.ascontiguousarray(
            lhsT.reshape(nk, 128, M).transpose(1, 0, 2).reshape(128, nk * M)
        ).astype(bf)

    Wp = in_proj_w * ln1_w[None, :]
    sb_full = in_proj_w @ ln1_b
    ln1b_nonzero = bool(np.any(sb_full != 0.0))

    Wfc = fc_w * ln2_w[None, :]
    sbfc_full = fc_w @ ln2_b + fc_b
    wfc_pack = pack_lhsT(np.ascontiguousarray(Wfc.T))
    wpj_pack = pack_lhsT(np.ascontiguousarray(proj_w.T))
    woT = np.ascontiguousarray(out_proj_w.T)                  # [DIN, E]
    wo_pack = np.ascontiguousarray(np.hstack(
        [woT[ci * 128:(ci + 1) * 128, :] for ci in range(16)])).astype(bf)
    sbfc_pack = np.ascontiguousarray(sbfc_full.reshape(32, 128).T).astype(f32)
    pjb_row = proj_b[None, :].astype(bf)

    A = -np.exp(A_log)

    per_core = []
    for c in range(NC):
        dsl = slice(c * DL, (c + 1) * DL)
        rows = np.concatenate([Wp[dsl], Wp[DIN + c * DL:DIN + (c + 1) * DL]])
        win_pack = pack_lhsT(np.ascontiguousarray(rows.T))
        sw_row = rows.sum(1)[None, :].astype(bf)
        sb_rows = np.concatenate([sb_full[dsl],
                                  sb_full[DIN + c * DL:DIN + (c + 1) * DL]])
        sb_pack = np.ascontiguousarray(sb_rows.reshape(4, 128).T).astype(f32)

        cw = conv_w[dsl, 0, :]
        convw_pack = np.ascontiguousarray(
            cw.reshape(2, 128, KC).transpose(1, 0, 2).reshape(128, 2 * KC)
        ).astype(f32)
        convb_pack = np.ascontiguousarray(
            conv_b[dsl].reshape(2, 128).T).astype(f32)

        xpw_pack = pack_lhsT(np.ascontiguousarray(x_proj_w[:, dsl].T))
        dtw_slice = np.ascontiguousarray(dt_proj_w[dsl].T).astype(bf)
        dtb_pack = np.ascontiguousarray(
            dt_proj_b[dsl].reshape(2, 128).T).astype(f32)
        asc_pack = np.ascontiguousarray(
            A[dsl].reshape(2, 128, NST).transpose(1, 0, 2).reshape(128, 2 * NST)
        ).astype(f32)
        dvec_pack = np.ascontiguousarray(D[dsl].reshape(2, 128).T).astype(f32)

        xresT_slice = np.ascontiguousarray(np.concatenate(
            [x2d[c * TOKB:(c + 1) * TOKB, :],
             x2d[L + c * TOKB:L + (c + 1) * TOKB, :]]))       # [256, E]

        per_core.append({
            "xT": xT_b, "win": win_pack, "sw_in": sw_row, "sb_in": sb_pack,
            "convw": convw_pack, "convb": convb_pack, "xpw": xpw_pack,
            "dtw": dtw_slice, "dtb": dtb_pack, "a_sc": asc_pack,
            "dvec": dvec_pack, "wo": wo_pack, "xresT": xresT_slice,
            "wfc": wfc_pack, "sbfc": sbfc_pack,
            "wpj": wpj_pack, "pjb_row": pjb_row,
            "ones128": np.ones((128, 1), bf),
            "ident": np.eye(128, dtype=bf),
        })
    return per_core, ln1b_nonzero


def kernel(**inputs):
    per_core, ln1b_nonzero = _prep_inputs(inputs)
    nc = _build(ln1b_nonzero)
    trace = bool(int(os.environ.get("BASSK_TRACE", "0")))
    try:
        res = run_bass_kernel_spmd(nc, per_core, core_ids=list(range(NC)),
                                   trace=trace)
    except Exception:
        # transient device hiccups (e.g. NRT exec-unit errors) clear on retry
        res = run_bass_kernel_spmd(nc, per_core, core_ids=list(range(NC)),
                                   trace=trace)
    kernel.last_results = res
    out2d = np.empty((TOK, E), np.float32)
    for c in range(NC):
        r = res.results[c]["outTT"]
        out2d[c * TOKB:(c + 1) * TOKB] = r[:TOKB]
        out2d[L + c * TOKB:L + (c + 1) * TOKB] = r[TOKB:]
    return out2d.reshape(B, L, E).astype(np.float32)


# revision 23
# speedup vs baseline: 1.1733x; 1.0492x over previous
"""Mamba block (dense_transformer nn_Block) on 8 Trainium2 NeuronCores.

Batch-half pipelined schedule. d_inner sharded 8-way (256 ch/core) for
in_proj/conv/scan; x_proj partials AllReduced per batch half; the scan output
is re-sharded to tokens by per-(d,half) AllToAlls; out_proj runs in a
token-transposed layout (tokens on partitions) so LN2 and its application are
pure scalar-engine work; fc consumes the transposed-back activations with
streamed weights and its output tiles feed proj directly as the stationary
operand.  The vector-engine scan of half b overlaps the tensor engine's
out_proj+MLP of half b-1.
"""
import os
import numpy as np
import ml_dtypes

import concourse.bass as bass
import concourse.bacc as bacc
import concourse.mybir as mybir
import concourse.tile as tile
from contextlib import ExitStack
from concourse.bass_utils import run_bass_kernel_spmd

BF16 = mybir.dt.bfloat16
F32 = mybir.dt.float32
AF = mybir.ActivationFunctionType
OP = mybir.AluOpType
bf = ml_dtypes.bfloat16

B, L, E = 2, 1024, 1024
DIN, NST, RDT, KC = 2 * E, 16, 64, 4
EPS = 1e-5
NC = 8
DL = DIN // NC          # 256 channels per core
TOK = B * L             # 2048
TOKB = 128              # tokens per core per batch half
HID = 4 * E             # 4096
G = 4                   # states per scan instruction
NG = NST // G

_BUILD_CACHE = {}


def _rep0(src_ap, parts=128):
    """Partition-broadcast: prepend a [0, parts] dim to an AP's pattern."""
    return bass.AP(src_ap.tensor, src_ap.offset,
                   [[0, parts]] + [list(p) for p in src_ap.ap])


def _build(ln1b_nonzero):
    key = (ln1b_nonzero,)
    if key in _BUILD_CACHE:
        return _BUILD_CACHE[key]

    nc = bacc.Bacc("TRN2", target_bir_lowering=False, debug=False, num_devices=NC)

    def din(name, shape, dt=BF16):
        return nc.dram_tensor(name, shape, dt, kind="ExternalInput").ap()

    xT = din("xT", [E, TOK])
    win = din("win", [128, 8 * 512])
    sw_in = din("sw_in", [1, 512])
    sb_in = din("sb_in", [128, 4], F32)
    convw = din("convw", [128, 2 * KC], F32)
    convb = din("convb", [128, 2], F32)
    xpw = din("xpw", [128, 2 * 96])
    dtw = din("dtw", [64, 256])
    dtb = din("dtb", [128, 2], F32)
    a_sc = din("a_sc", [128, 2 * NST], F32)
    dvec = din("dvec", [128, 2], F32)
    wo = din("wo", [128, 16 * 1024])
    xresT = din("xresT", [2 * TOKB, E], F32)
    wfc = din("wfc", [128, 8 * HID])
    sbfc_row = din("sbfc_row", [1, HID])
    wpj = din("wpj", [128, 32 * E])
    pjb_row = din("pjb_row", [1, E])
    ones128 = din("ones128", [128, 1])
    ident = din("ident", [128, 128])

    outTT = nc.dram_tensor("outTT", [2 * TOKB, E], F32, kind="ExternalOutput").ap()

    cc_dummy_in = nc.dram_tensor("cc_dummy_in", [1, 16], F32)
    cc_dummy_out = nc.dram_tensor("cc_dummy_out", [1, 16], F32, addr_space="Shared")
    ar_ins = [nc.dram_tensor(f"ar_in{b}", [96, L], BF16) for b in range(2)]
    ar_outs = [nc.dram_tensor(f"ar_out{b}", [96, L], BF16, addr_space="Shared")
               for b in range(2)]
    bc_bfs = [nc.dram_tensor(f"bc_bf{b}", [32, L], BF16) for b in range(2)]
    a2a_ins = [nc.dram_tensor(f"a2a_in{b}", [NC, 2 * 128 * TOKB], BF16)
               for b in range(2)]
    a2a_outs = [nc.dram_tensor(f"a2a_out{b}", [NC, 2 * 128 * TOKB], BF16)
                for b in range(2)]
    RG = [list(range(NC))]

    with tile.TileContext(nc) as tc, ExitStack() as _stk:
        # warm the collective stream early (absorbs ~80us barrier + delay)
        nc.gpsimd.collective_compute("AllReduce", OP.add, ins=[cc_dummy_in[:]],
                                     outs=[cc_dummy_out[:]], replica_groups=RG)

        cpool = _stk.enter_context(tc.tile_pool(name="consts", bufs=1))
        ones_t = cpool.tile([128, 1], BF16, tag="ones")
        nc.sync.dma_start(ones_t[:], ones128[:])
        ident_t = cpool.tile([128, 128], BF16, tag="ident")
        nc.sync.dma_start(ident_t[:], ident[:])
        ones_row = cpool.tile([1, 128], BF16, tag="onesrow")
        nc.sync.dma_start(ones_row[:], ones128[:].rearrange("p q -> q p"))
        ones_row_f = cpool.tile([1, 128], F32, tag="onesrowf")
        nc.vector.tensor_copy(ones_row_f[:], ones_row[:])
        sw_t = cpool.tile([1, 512], BF16, tag="sw")
        nc.sync.dma_start(sw_t[:], sw_in[:])
        convw_t = cpool.tile([128, 2 * KC], F32, tag="convw")
        nc.sync.dma_start(convw_t[:], convw[:])
        convb_t = cpool.tile([128, 2], F32, tag="convb")
        nc.sync.dma_start(convb_t[:], convb[:])
        xpw_t = cpool.tile([128, 2 * 96], BF16, tag="xpw")
        nc.sync.dma_start(xpw_t[:], xpw[:])
        dtw_t = cpool.tile([64, 256], BF16, tag="dtw")
        nc.sync.dma_start(dtw_t[:], dtw[:])
        dtb_t = cpool.tile([128, 2], F32, tag="dtb")
        nc.sync.dma_start(dtb_t[:], dtb[:])
        asc_t = cpool.tile([128, 2 * NST], F32, tag="asc")
        nc.sync.dma_start(asc_t[:], a_sc[:])
        dvec_t = cpool.tile([128, 2], F32, tag="dvec")
        nc.sync.dma_start(dvec_t[:], dvec[:])
        sbfc_t = cpool.tile([1, HID], BF16, tag="sbfc")
        nc.sync.dma_start(sbfc_t[:], sbfc_row[:])
        pjb_t = cpool.tile([1, E], BF16, tag="pjb")
        nc.sync.dma_start(pjb_t[:], pjb_row[:])
        eps_t = cpool.tile([128, 1], F32, tag="eps")
        nc.vector.memset(eps_t[:], EPS)
        sbin_t = cpool.tile([128, 4], F32, tag="sbin")
        if ln1b_nonzero:
            nc.sync.dma_start(sbin_t[:], sb_in[:])
        xresT_t = [cpool.tile([TOKB, E], F32, tag=f"xresT{b}",
                              name=f"xresT_t{b}") for b in range(2)]
        for b in range(2):
            nc.sync.dma_start(xresT_t[b][:], xresT[b * TOKB:(b + 1) * TOKB, :])

        # ---- long-lived pools (to program end) ----
        xmp = [[None, None], [None, None]]
        zt = [[None, None], [None, None]]
        mbp1 = _stk.enter_context(tc.tile_pool(name="mamba1", bufs=1))
        pa = _stk.enter_context(tc.tile_pool(name="scan_a", bufs=3))
        pbh = _stk.enter_context(tc.tile_pool(name="scan_bh", bufs=3))
        pr = _stk.enter_context(tc.tile_pool(name="scan_r", bufs=2))
        py = _stk.enter_context(tc.tile_pool(name="scan_y", bufs=2))
        ps_y = _stk.enter_context(tc.tile_pool(name="ps_y", bufs=1, space="PSUM"))
        # ---- short-lived pools (close at mid2 start) ----
        _mb0stk = ExitStack()
        mbp0 = _mb0stk.enter_context(tc.tile_pool(name="mamba0", bufs=1))
        _iostk = ExitStack()
        iop = _iostk.enter_context(tc.tile_pool(name="mamba_io", bufs=1))
        mbp = [mbp0, mbp1]
        zs = [[mbp[b].tile([128, L], BF16, tag=f"zs{d}", name=f"zs{d}{b}")
               for b in range(2)] for d in range(2)]
        xs = [[mbp[b].tile([128, L], BF16, tag=f"xs{d}", name=f"xs{d}{b}")
               for b in range(2)] for d in range(2)]
        dt_t = [[mbp[b].tile([128, L], BF16, tag=f"dt{d}", name=f"dt{d}{b}")
                 for b in range(2)] for d in range(2)]
        dtx = [[mbp[b].tile([128, L], BF16, tag=f"dtx{d}", name=f"dtx{d}{b}")
                for b in range(2)] for d in range(2)]

        # ---- phase-A pools (head + mid1) ----
        _astk = ExitStack()
        p1 = _astk.enter_context(tc.tile_pool(name="ph1", bufs=1))
        p1sq = _astk.enter_context(tc.tile_pool(name="ph1sq", bufs=3))
        ps_st = _astk.enter_context(tc.tile_pool(name="ps_st", bufs=1, space="PSUM"))
        ps_in = _astk.enter_context(tc.tile_pool(name="ps_in", bufs=2, space="PSUM"))
        ps_rb = _astk.enter_context(tc.tile_pool(name="ps_rb", bufs=1, space="PSUM"))
        cvp = _astk.enter_context(tc.tile_pool(name="conv", bufs=2))
        xpp = _astk.enter_context(tc.tile_pool(name="xp", bufs=2))
        ps_xp = _astk.enter_context(tc.tile_pool(name="ps_xp", bufs=1, space="PSUM"))
        dts = _astk.enter_context(tc.tile_pool(name="dts", bufs=2))

        xt = [p1.tile([128, TOK], BF16, tag=f"xt{k}", name=f"xt{k}")
              for k in range(8)]
        for k in range(8):
            nc.sync.dma_start(xt[k][:], xT[k * 128:(k + 1) * 128, :])
        win_t = p1.tile([128, 8 * 512], BF16, tag="win")
        nc.sync.dma_start(win_t[:], win[:])
        negs = [p1.tile([1, L], BF16, tag=f"negm{b}", name=f"negm{b}")
                for b in range(2)]
        r_reps = [p1.tile([128, L], BF16, tag="r_rep", name=f"r_rep{b}")
                  for b in range(2)]

        def phase_A1(b):
            """LN1 stats, in_proj xm tiles, conv+silu for half b."""
            for d in range(2):
                xmp[d][b] = iop.tile([128, 3 + L], BF16, tag=f"xmp{d}",
                                     name=f"xmp{d}{b}")
                nc.vector.memset(xmp[d][b][:, 0:3], 0.0)
                zt[d][b] = iop.tile([128, L], BF16, tag=f"z{d}",
                                    name=f"zt{d}{b}")
            # ---- LN1 stats ----
            sum_sb = p1.tile([1, L], F32, tag="rows", bufs=3)
            sq_sb = p1.tile([1, L], F32, tag="rows", bufs=3)
            for ch in range(2):
                sl = slice(b * L + ch * 512, b * L + (ch + 1) * 512)
                dsl = slice(ch * 512, (ch + 1) * 512)
                pss = ps_st.tile([1, 512], F32, tag="pstat", bufs=2)
                for k in range(8):
                    nc.tensor.matmul(pss[:], ones_t[:], xt[k][:, sl],
                                     start=(k == 0), stop=(k == 7))
                nc.vector.tensor_copy(sum_sb[:, dsl], pss[:])
                psq = ps_st.tile([1, 512], F32, tag="pstat", bufs=2)
                for k in range(8):
                    xq = p1sq.tile([128, 512], BF16, tag="xq", bufs=2)
                    nc.scalar.activation(xq[:], xt[k][:, sl], AF.Square)
                    nc.tensor.matmul(psq[:], ones_t[:], xq[:],
                                     start=(k == 0), stop=(k == 7))
                nc.vector.tensor_copy(sq_sb[:, dsl], psq[:])
            m_neg = p1.tile([1, L], F32, tag="rows", bufs=3)
            nc.vector.tensor_scalar_mul(m_neg[:], sum_sb[:], -1.0 / E)
            nc.vector.tensor_copy(negs[b][:], m_neg[:])
            msq = p1.tile([1, L], F32, tag="rows", bufs=3)
            nc.vector.tensor_tensor(msq[:], m_neg[:], m_neg[:], OP.mult)
            var = p1.tile([1, L], F32, tag="rows", bufs=3)
            nc.vector.scalar_tensor_tensor(var[:], sq_sb[:], 1.0 / E,
                                           msq[:], OP.mult, OP.subtract)
            lnv = p1.tile([1, L], F32, tag="rows", bufs=3)
            nc.scalar.activation(lnv[:], var[:], AF.Ln, bias=eps_t[0:1, :])
            r_sb = p1.tile([1, L], F32, tag="rows", bufs=3)
            nc.scalar.activation(r_sb[:], lnv[:], AF.Exp, scale=-0.5)
            for hh in range(2):
                prb = ps_rb.tile([128, 512], F32, tag="prb")
                nc.tensor.matmul(prb[:], ones_row_f[:],
                                 r_sb[:, hh * 512:(hh + 1) * 512],
                                 start=True, stop=True)
                nc.vector.tensor_copy(r_reps[b][:, hh * 512:(hh + 1) * 512],
                                      prb[:])

            # ---- in_proj (xm tiles first, z tiles after conv) ----
            def inproj_tile(mt, ch):
                col = ch * 512
                sl = slice(b * L + col, b * L + col + 512)
                ps = ps_in.tile([128, 512], F32, tag="ps", bufs=2)
                for k in range(8):
                    nc.tensor.matmul(
                        ps[:],
                        win_t[:, k * 512 + mt * 128:k * 512 + (mt + 1) * 128],
                        xt[k][:, sl], start=(k == 0), stop=False)
                nc.tensor.matmul(ps[:], sw_t[:, mt * 128:(mt + 1) * 128],
                                 negs[b][:, col:col + 512],
                                 start=False, stop=True)
                if mt < 2:
                    dst = xmp[mt][b][:, 3 + col:3 + col + 512]
                else:
                    dst = zt[mt - 2][b][:, col:col + 512]
                if ln1b_nonzero:
                    tmp = p1sq.tile([128, 512], F32, tag="eptmp")
                    nc.vector.tensor_tensor(tmp[:], ps[:],
                                            r_reps[b][:, col:col + 512],
                                            OP.mult)
                    nc.scalar.activation(dst, tmp[:], AF.Identity,
                                         bias=sbin_t[:, mt:mt + 1])
                else:
                    nc.vector.tensor_tensor(dst, ps[:],
                                            r_reps[b][:, col:col + 512],
                                            OP.mult)

            for mt in range(2):
                for ch in range(2):
                    inproj_tile(mt, ch)

            # ---- conv + silu ----
            for d in range(2):
                acc0 = cvp.tile([128, L], BF16, tag="acc", bufs=2)
                nc.vector.tensor_scalar_mul(acc0[:], xmp[d][b][:, 0:L],
                                            convw_t[:, d * KC:d * KC + 1])
                for k in (1, 2, 3):
                    acc1 = cvp.tile([128, L], BF16, tag="acc", bufs=2)
                    nc.vector.scalar_tensor_tensor(
                        acc1[:], xmp[d][b][:, k:k + L],
                        convw_t[:, d * KC + k:d * KC + k + 1],
                        acc0[:], OP.mult, OP.add)
                    acc0 = acc1
                nc.scalar.activation(xs[d][b][:], acc0[:], AF.Silu,
                                     bias=convb_t[:, d:d + 1])

            phase_A1._tail[b] = inproj_tile

        phase_A1._tail = [None, None]

        def phase_A2b(b):
            """in_proj z tiles + silu(z)."""
            inproj_tile = phase_A1._tail[b]
            for mt in range(2, 4):
                for ch in range(2):
                    inproj_tile(mt, ch)
            for d in range(2):
                nc.scalar.activation(zs[d][b][:], zt[d][b][:], AF.Silu)

        def phase_A2a(b):
            # ---- x_proj partial + AllReduce ----
            xdblp = xpp.tile([96, L], BF16, tag="xdblp", bufs=1)
            for ch in range(2):
                col = ch * 512
                psx = ps_xp.tile([96, 512], F32, tag="psx")
                for k in range(2):
                    nc.tensor.matmul(psx[:], xpw_t[:, k * 96:(k + 1) * 96],
                                     xs[k][b][:, col:col + 512],
                                     start=(k == 0), stop=(k == 1))
                nc.vector.tensor_copy(xdblp[:, col:col + 512], psx[:])
            nc.sync.dma_start(ar_ins[b][:], xdblp[:])
            nc.gpsimd.collective_compute("AllReduce", OP.add,
                                         ins=[ar_ins[b][:]],
                                         outs=[ar_outs[b][:]],
                                         replica_groups=RG)

        def phase_dt(b):
            """dt softplus + dtx for half b (after AR(b))."""
            dtr_b = dts.tile([64, L], BF16, tag="dtrb", bufs=1)
            nc.gpsimd.dma_start(dtr_b[:], ar_outs[b][0:64, :])
            nc.gpsimd.dma_start(bc_bfs[b][:], ar_outs[b][64:96, :])
            dtes = []
            for mt in range(2):
                dte_t = dts.tile([128, L], BF16, tag="dte")
                for ch in range(2):
                    col = ch * 512
                    psd = ps_in.tile([128, 512], F32, tag="ps", bufs=2)
                    nc.tensor.matmul(psd[:],
                                     dtw_t[:, mt * 128:(mt + 1) * 128],
                                     dtr_b[:, col:col + 512],
                                     start=True, stop=True)
                    nc.scalar.activation(dte_t[:, col:col + 512],
                                         psd[:], AF.Exp,
                                         bias=dtb_t[:, mt:mt + 1])
                dtes.append(dte_t)
            for mt in range(2):
                nc.scalar.activation(dt_t[mt][b][:], dtes[mt][:],
                                     AF.Ln, bias=1.0)
                nc.vector.tensor_tensor(dtx[mt][b][:], dt_t[mt][b][:],
                                        xs[mt][b][:], OP.mult)

        psy_cur = [None]

        def scan_group(d, b, g):
            if g == 0:
                psy_cur[0] = ps_y.tile([128, L], F32, tag="psy", name="psy")
            psy = psy_cur[0]
            a_t = pa.tile([128, G, L], BF16, tag="a")
            for j in range(G):
                n = g * G + j
                nc.scalar.activation(
                    a_t[:, j, :], dt_t[d][b][:], AF.Exp,
                    scale=asc_t[:, d * NST + n:d * NST + n + 1])
            nc.vector.memset(a_t[:, :, 0:1], 0.0)
            brep = pr.tile([128, G, L], BF16, tag="bcr")
            nc.sync.dma_start(brep[:],
                              _rep0(bc_bfs[b][g * G:(g + 1) * G, :]))
            bx = pbh.tile([128, G, L], BF16, tag="bxhc")
            dslice = dtx[d][b][:]
            dxb = bass.AP(dslice.tensor, dslice.offset,
                          [list(dslice.ap[0]), [0, G], [1, L]])
            nc.vector.tensor_tensor(bx[:], dxb, brep[:], OP.mult)
            h_t = pbh.tile([128, G, L], BF16, tag="bxhc")
            nc.vector.tensor_tensor_scan(
                h_t[:].rearrange("p a b -> p (a b)"),
                a_t[:].rearrange("p a b -> p (a b)"),
                bx[:].rearrange("p a b -> p (a b)"),
                0.0, OP.mult, OP.add)
            crep = pr.tile([128, G, L], BF16, tag="bcr")
            nc.sync.dma_start(crep[:],
                              _rep0(bc_bfs[b][16 + g * G:16 + (g + 1) * G, :]))
            hc = pbh.tile([128, G, L], BF16, tag="bxhc")
            nc.vector.tensor_tensor(hc[:], h_t[:], crep[:], OP.mult)
            for j in range(G):
                for hh in range(2):
                    nc.tensor.matmul(
                        psy[:, hh * 512:(hh + 1) * 512], ident_t[:],
                        hc[:, j, hh * 512:(hh + 1) * 512],
                        start=(g == 0 and j == 0),
                        stop=(g == NG - 1 and j == G - 1))

        def scan_fin(d, b):
            """y2/y3 + A2A staging + launch A2A(d, b)."""
            psy = psy_cur[0]
            y2 = py.tile([128, L], BF16, tag="y2")
            nc.vector.scalar_tensor_tensor(y2[:], xs[d][b][:],
                                           dvec_t[:, d:d + 1], psy[:],
                                           OP.mult, OP.add)
            y3 = py.tile([128, L], BF16, tag="y2")
            nc.vector.tensor_tensor(y3[:], y2[:], zs[d][b][:], OP.mult)
            # single strided DMA stages all 8 destination rows
            seg = a2a_ins[b][:, d * 128 * TOKB:(d + 1) * 128 * TOKB]
            nc.sync.dma_start(
                seg.rearrange("c (p q) -> p c q", p=128),
                y3[:].rearrange("p (c q) -> p c q", c=NC))
            if d == 1:
                nc.gpsimd.collective_compute("AllToAll", OP.bypass,
                                             ins=[a2a_ins[b][:]],
                                             outs=[a2a_outs[b][:]],
                                             replica_groups=RG)

        # ================= emission: head =================
        phase_A1(0)
        phase_A2a(0)    # AR(b0) launches here
        phase_A2b(0)
        phase_A1(1)     # tensor/vector fill the AR(b0) wait
        phase_dt(0)
        phase_A2a(1)    # AR(b1) launches here
        phase_A2b(1)

        # ================= emission: mid1 (scan b0) =================
        scan_group(0, 0, 0)
        scan_group(0, 0, 1)
        scan_group(0, 0, 2)
        scan_group(0, 0, 3)
        scan_fin(0, 0)
        phase_dt(1)
        scan_group(1, 0, 0)
        scan_group(1, 0, 1)
        scan_group(1, 0, 2)
        scan_group(1, 0, 3)
        scan_fin(1, 0)

        _astk.close()   # frees xt/win pools + phase-A PSUM
        _iostk.close()  # frees xmp/zt rings
        _mb0stk.close()  # frees b0 scan activations

        # MLP-side pools + weight tiles (wo resident; wfc/wpj streamed)
        mlpp = _stk.enter_context(tc.tile_pool(name="mlp", bufs=1, side="right"))
        wo_t = mlpp.tile([128, 16 * 1024], BF16, tag="wo")
        nc.sync.dma_start(wo_t[:], wo[:])

        opool = _stk.enter_context(tc.tile_pool(name="opool", bufs=1))
        wfcp = _stk.enter_context(tc.tile_pool(name="wfcp", bufs=2, side="right"))
        wpjp = _stk.enter_context(tc.tile_pool(name="wpjp", bufs=2, side="right"))

        def phase_O(b, interleave=None):
            """out_proj + LN2 + MLP for half b's 128 tokens.

            interleave: dict step-name -> fn emitting scan work between
            tensor-heavy steps.
            """
            steps = dict(interleave or {})

            def run(tag):
                if tag in steps:
                    steps.pop(tag)()

            with tc.tile_pool(name="ps_op", bufs=1, space="PSUM") as ps_op, \
                 tc.tile_pool(name="yfp", bufs=1) as yfp:
                r1T_ps = ps_op.tile([128, E], F32, tag="r1T")
                yf_t = yfp.tile([128, 2 * NC * TOKB], BF16, tag="yf",
                                name=f"yf_{b}")
                for d in range(2):
                    seg = a2a_outs[b][:, d * 128 * TOKB:(d + 1) * 128 * TOKB]
                    nc.sync.dma_start(
                        yf_t[:, d * NC * TOKB:(d + 1) * NC * TOKB]
                        .rearrange("p (i q) -> p i q", i=NC),
                        seg.rearrange("i (p q) -> p i q", p=128))
                run("yf")
                for d in range(2):
                    for i in range(NC):
                        ci = i * 2 + d
                        for hh in range(2):
                            nc.tensor.matmul(
                                r1T_ps[:, hh * 512:(hh + 1) * 512],
                                yf_t[:, (d * NC + i) * TOKB:
                                     (d * NC + i + 1) * TOKB],
                                wo_t[:, ci * 1024 + hh * 512:
                                     ci * 1024 + (hh + 1) * 512],
                                start=(d == 0 and i == 0),
                                stop=(d == 1 and i == NC - 1),
                                skip_group_check=True)
                run("op")
                r1fT = opool.tile([128, E], F32, tag="r1fT", bufs=1,
                                  name=f"r1fT{b}")
                nc.vector.tensor_tensor(r1fT[:], r1T_ps[:], xresT_t[b][:],
                                        OP.add)

            # LN2 stats on scalar engine (per-partition = per-token)
            scr = opool.tile([128, E], F32, tag="scr", bufs=1)
            s1 = opool.tile([128, 1], F32, tag="s1", bufs=2)
            s2 = opool.tile([128, 1], F32, tag="s2", bufs=2)
            nc.scalar.activation(scr[:], r1fT[:], AF.Identity, accum_out=s1[:])
            nc.scalar.activation(scr[:], r1fT[:], AF.Square, accum_out=s2[:])
            m_neg = opool.tile([128, 1], F32, tag="mneg", bufs=2)
            nc.vector.tensor_scalar_mul(m_neg[:], s1[:], -1.0 / E)
            msq = opool.tile([128, 1], F32, tag="msq", bufs=2)
            nc.vector.tensor_tensor(msq[:], m_neg[:], m_neg[:], OP.mult)
            var = opool.tile([128, 1], F32, tag="var", bufs=2)
            nc.vector.scalar_tensor_tensor(var[:], s2[:], 1.0 / E, msq[:],
                                           OP.mult, OP.subtract)
            lnv = opool.tile([128, 1], F32, tag="lnv", bufs=2)
            nc.scalar.activation(lnv[:], var[:], AF.Ln, bias=eps_t[:])
            rstd = opool.tile([128, 1], F32, tag="rstd", bufs=2)
            nc.scalar.activation(rstd[:], lnv[:], AF.Exp, scale=-0.5)
            nmb = opool.tile([128, 1], F32, tag="nmb", bufs=2)
            nc.vector.tensor_tensor(nmb[:], m_neg[:], rstd[:], OP.mult)
            r1nT = opool.tile([128, E], BF16, tag="r1nT", bufs=1,
                              name=f"r1nT{b}")
            nc.scalar.activation(r1nT[:], r1fT[:], AF.Identity,
                                 bias=nmb[:], scale=rstd[:])
            # transpose r1nT chunks -> [e, tok] stationary for fc
            # (tensor-engine transpose + scalar PSUM->SBUF copy)
            r1tt = opool.tile([128, E], BF16, tag="r1tt", bufs=1,
                              name=f"r1tt{b}")
            with tc.tile_pool(name="ps_tr1", bufs=2, space="PSUM") as ps_tr1:
                for k in range(8):
                    ptr = ps_tr1.tile([128, 128], BF16, tag="ptr", bufs=2)
                    nc.tensor.transpose(ptr[:], r1nT[:, k * 128:(k + 1) * 128],
                                        ident_t[:])
                    nc.scalar.copy(r1tt[:, k * 128:(k + 1) * 128], ptr[:])
            run("r1")

            # fc (tokens on partitions): stationary = transposed
            # activations (16 LDW total), moving = streamed wfc slabs.
            # gelu'd output h1sT is transposed back per 512-hid piece on the
            # tensor engine so proj can consume [hid, tok] stationary tiles.
            h1sT = opool.tile([128, HID], BF16, tag="h1s", bufs=1,
                              name=f"h1sT{b}")
            h1tt = opool.tile([128, HID], BF16, tag="h1t", bufs=1,
                              name=f"h1tt{b}")
            with tc.tile_pool(name="ps_fc", bufs=2, space="PSUM") as ps_fc, \
                 tc.tile_pool(name="ps_tr", bufs=2, space="PSUM") as ps_tr:
                for q2 in range(8):
                    wslab = wfcp.tile([128, 8 * 512], BF16, tag="wfc",
                                      name=f"wfc{b}_{q2}")
                    nc.gpsimd.dma_start(
                        wslab[:].rearrange("p (k m) -> p k m", k=8),
                        bass.AP(wfc.tensor, wfc.offset + q2 * 512,
                                [list(wfc.ap[0]), [HID, 8], [1, 512]]))
                    psf = ps_fc.tile([128, 512], F32, tag="psf", bufs=2)
                    for k in range(8):
                        nc.tensor.matmul(psf[:],
                                         r1tt[:, k * 128:(k + 1) * 128],
                                         wslab[:, k * 512:(k + 1) * 512],
                                         start=(k == 0), stop=False)
                    nc.tensor.matmul(psf[:], ones_row[:],
                                     sbfc_t[:, q2 * 512:(q2 + 1) * 512],
                                     start=False, stop=True)
                    nc.scalar.activation(h1sT[:, q2 * 512:(q2 + 1) * 512],
                                         psf[:], AF.Gelu)
                    for u in range(4):
                        j = q2 * 4 + u
                        ptr = ps_tr.tile([128, 128], BF16, tag="ptr2", bufs=2)
                        nc.tensor.transpose(
                            ptr[:], h1sT[:, j * 128:(j + 1) * 128], ident_t[:])
                        nc.scalar.copy(h1tt[:, j * 128:(j + 1) * 128], ptr[:])
                    if q2 % 2 == 1:
                        run(f"fc{q2 // 2}")

            # proj: stationary = h1 tiles; moving = streamed wpj slabs
            with tc.tile_pool(name="ps_pj", bufs=1, space="PSUM") as ps_pj:
                psp = ps_pj.tile([128, E], F32, tag="psp")
                for jg in range(8):
                    wpj_g = wpjp.tile([128, 4 * E], BF16, tag="wpjg",
                                      name=f"wpj{b}_{jg}")
                    nc.gpsimd.dma_start(
                        wpj_g[:], wpj[:, jg * 4 * E:(jg + 1) * 4 * E])
                    for jj in range(4):
                        j = jg * 4 + jj
                        for hh in range(2):
                            nc.tensor.matmul(
                                psp[:, hh * 512:(hh + 1) * 512],
                                h1tt[:, j * 128:(j + 1) * 128],
                                wpj_g[:, jj * E + hh * 512:
                                      jj * E + (hh + 1) * 512],
                                start=(j == 0), stop=False,
                                skip_group_check=True)
                    if jg % 2 == 1:
                        run(f"pj{jg // 2}")
                for hh in range(2):
                    nc.tensor.matmul(psp[:, hh * 512:(hh + 1) * 512],
                                     ones_row[:],
                                     pjb_t[:, hh * 512:(hh + 1) * 512],
                                     start=False, stop=True,
                                     skip_group_check=True)
                ot = opool.tile([128, E], F32, tag="ot", bufs=1)
                nc.vector.tensor_tensor(ot[:], psp[:], r1fT[:], OP.add)
                nc.sync.dma_start(outTT[b * TOKB:(b + 1) * TOKB, :], ot[:])
            # any steps not yet consumed
            for tag in list(steps):
                steps.pop(tag)()

        # ================= emission: mid2 (scan b1 || O(b0)) =================
        sg = scan_group
        phase_O(0, interleave={
            "yf": lambda: sg(0, 1, 0),
            "op": lambda: sg(0, 1, 1),
            "r1": lambda: sg(0, 1, 2),
            "fc0": lambda: (sg(0, 1, 3), scan_fin(0, 1)),
            "fc1": lambda: sg(1, 1, 0),
            "fc2": lambda: sg(1, 1, 1),
            "fc3": lambda: sg(1, 1, 2),
            "pj0": lambda: (sg(1, 1, 3), scan_fin(1, 1)),
        })

        # ================= emission: tail (O(b1)) =================
        phase_O(1)

    nc.compile()
    _BUILD_CACHE[key] = nc
    return nc


def _prep_inputs(inputs):
    """Host-side sharding/packing. Returns list of per-core input dicts."""
    f32 = np.float32
    x = np.asarray(inputs["x"], f32)
    ln1_w = np.asarray(inputs["ln1_w"], f32)
    ln1_b = np.asarray(inputs["ln1_b"], f32)
    in_proj_w = np.asarray(inputs["in_proj_w"], f32)
    conv_w = np.asarray(inputs["conv_w"], f32)
    conv_b = np.asarray(inputs["conv_b"], f32)
    x_proj_w = np.asarray(inputs["x_proj_w"], f32)
    dt_proj_w = np.asarray(inputs["dt_proj_w"], f32)
    dt_proj_b = np.asarray(inputs["dt_proj_b"], f32)
    A_log = np.asarray(inputs["A_log"], f32)
    D = np.asarray(inputs["D"], f32)
    out_proj_w = np.asarray(inputs["out_proj_w"], f32)
    ln2_w = np.asarray(inputs["ln2_w"], f32)
    ln2_b = np.asarray(inputs["ln2_b"], f32)
    fc_w = np.asarray(inputs["fc_w"], f32)
    fc_b = np.asarray(inputs["fc_b"], f32)
    proj_w = np.asarray(inputs["proj_w"], f32)
    proj_b = np.asarray(inputs["proj_b"], f32)

    x2d = np.ascontiguousarray(x.reshape(TOK, E))             # [TOK, E]
    xT_b = np.ascontiguousarray(x2d.T).astype(bf)             # [E, TOK]

    def pack_lhsT(lhsT):
        K, M = lhsT.shape
        nk = K // 128
        return np.ascontiguousarray(
            lhsT.reshape(nk, 128, M).transpose(1, 0, 2).reshape(128, nk * M)
        ).astype(bf)

    Wp = in_proj_w * ln1_w[None, :]
    sb_full = in_proj_w @ ln1_b
    ln1b_nonzero = bool(np.any(sb_full != 0.0))

    Wfc = fc_w * ln2_w[None, :]
    sbfc_full = fc_w @ ln2_b + fc_b
    wfc_pack = pack_lhsT(np.ascontiguousarray(Wfc.T))
    wpj_pack = pack_lhsT(np.ascontiguousarray(proj_w.T))
    woT = np.ascontiguousarray(out_proj_w.T)                  # [DIN, E]
    wo_pack = np.ascontiguousarray(np.hstack(
        [woT[ci * 128:(ci + 1) * 128, :] for ci in range(16)])).astype(bf)
    sbfc_row = sbfc_full[None, :].astype(bf)
    pjb_row = proj_b[None, :].astype(bf)

    A = -np.exp(A_log)

    per_core = []
    for c in range(NC):
        dsl = slice(c * DL, (c + 1) * DL)
        rows = np.concatenate([Wp[dsl], Wp[DIN + c * DL:DIN + (c + 1) * DL]])
        win_pack = pack_lhsT(np.ascontiguousarray(rows.T))
        sw_row = rows.sum(1)[None, :].astype(bf)
        sb_rows = np.concatenate([sb_full[dsl],
                                  sb_full[DIN + c * DL:DIN + (c + 1) * DL]])
        sb_pack = np.ascontiguousarray(sb_rows.reshape(4, 128).T).astype(f32)

        cw = conv_w[dsl, 0, :]
        convw_pack = np.ascontiguousarray(
            cw.reshape(2, 128, KC).transpose(1, 0, 2).reshape(128, 2 * KC)
        ).astype(f32)
        convb_pack = np.ascontiguousarray(
            conv_b[dsl].reshape(2, 128).T).astype(f32)

        xpw_pack = pack_lhsT(np.ascontiguousarray(x_proj_w[:, dsl].T))
        dtw_slice = np.ascontiguousarray(dt_proj_w[dsl].T).astype(bf)
        dtb_pack = np.ascontiguousarray(
            dt_proj_b[dsl].reshape(2, 128).T).astype(f32)
        asc_pack = np.ascontiguousarray(
            A[dsl].reshape(2, 128, NST).transpose(1, 0, 2).reshape(128, 2 * NST)
        ).astype(f32)
        dvec_pack = np.ascontiguousarray(D[dsl].reshape(2, 128).T).astype(f32)

        xresT_slice = np.ascontiguousarray(np.concatenate(
            [x2d[c * TOKB:(c + 1) * TOKB, :],
             x2d[L + c * TOKB:L + (c + 1) * TOKB, :]]))       # [256, E]

        per_core.append({
            "xT": xT_b, "win": win_pack, "sw_in": sw_row, "sb_in": sb_pack,
            "convw": convw_pack, "convb": convb_pack, "xpw": xpw_pack,
            "dtw": dtw_slice, "dtb": dtb_pack, "a_sc": asc_pack,
            "dvec": dvec_pack, "wo": wo_pack, "xresT": xresT_slice,
            "wfc": wfc_pack, "sbfc_row": sbfc_row,
            "wpj": wpj_pack, "pjb_row": pjb_row,
            "ones128": np.ones((128, 1), bf),
            "ident": np.eye(128, dtype=bf),
        })
    return per_core, ln1b_nonzero


def kernel(**inputs):
    per_core, ln1b_nonzero = _prep_inputs(inputs)
    nc = _build(ln1b_nonzero)
    trace = bool(int(os.environ.get("BASSK_TRACE", "0")))
    try:
        res = run_bass_kernel_spmd(nc, per_core, core_ids=list(range(NC)),
                                   trace=trace)
    except Exception:
        # transient device hiccups (e.g. NRT exec-unit errors) clear on retry
        res = run_bass_kernel_spmd(nc, per_core, core_ids=list(range(NC)),
                                   trace=trace)
    kernel.last_results = res
    out2d = np.empty((TOK, E), np.float32)
    for c in range(NC):
        r = res.results[c]["outTT"]
        out2d[c * TOKB:(c + 1) * TOKB] = r[:TOKB]
        out2d[L + c * TOKB:L + (c + 1) * TOKB] = r[TOKB:]
    return out2d.reshape(B, L, E).astype(np.float32)
